# revision 41
# baseline (speedup 1.0000x reference)
"""Trainium2 Bass kernel for multi-head attention (B=2, P=2048, M=1024, N=16, H=64).

out = softmax(mask(x@Wq @ (x@Wk)^T / sqrt(H))) @ (x@Wv) @ Wo + biases,
with the module's strictly-upper-triangular keep mask (row P-1 fully masked).

Sharding: 8 cores = 2 batches x 4 head-groups. Core c handles batch c//4,
heads [4*(c%4), 4*(c%4)+4); the host sums the 4 partial output projections
per batch and patches the fully-masked query row P-1 analytically.

v3 design (fp8 DoubleRow where precision allows, ACT-exp-bound):
  - q/k projections (K=1024) run as fp8e4 DoubleRow matmuls (2 K-chunks
    per pass, 0.5 cyc/row). The v projection runs in bf16: fp8 v errors
    hit concentrated-attention rows at full strength (measured).
  - z = v_aug @ exp keeps DoubleRow speed at 16-bit-grade v precision:
    v_aug^T is stored as an fp8 hi+lo residual pair (v = hi + lo + O(e^2))
    in the two DR planes, and the fp8 ex operand is duplicated across
    planes with a stride-0 AP. Output projection runs in fp16.
  - Scores stay bf16, two heads row-packed in disjoint PE row groups.
  - The triangular mask is applied ADDITIVELY on the PE (eye @ maskneg
    accumulated into score PSUM): exp then yields exact fp8 zeros, so no
    DVE masking and no garbage in the DoubleRow-widened columns.
  - Softmax denominators come from an appended ones column in v_aug^T;
    normalization = DVE reciprocal of the denom row -> gpsimd
    partition_broadcast -> one DVE multiply straight into the fp8 zp tile
    (replaces the baseline's 4 PE transposes per chunk).
  - ACT runs exp only (one activation per pk chunk covers both heads);
    all PSUM evacuation is on DVE; DMA issue on the sync queue.
  - x DMA is chunked/ordered so the first projection starts ~1us in.
  - Timeline: proj(pr0) -> attention(pr0, j=3..0) with proj(pr1)+v1
    transposes interleaved into PE slack -> attention(pr1) with the
    output projection per j trailing.
"""
import sys

import numpy as np

if "/opt/trn_rl_repo" not in sys.path:
    sys.path.insert(0, "/opt/trn_rl_repo")

import concourse.bacc as bacc
import concourse.tile as tile
from concourse import mybir
from concourse import bass_utils
import ml_dtypes

B, P, M, N, H = 2, 2048, 1024, 16, 64
NCORES = 8
HPC = 4          # heads per core
NPAIRS = 2       # head pairs per core
MKD = 4          # DoubleRow contraction chunks (256 each) for projections
PT = P // 512    # 4 pq tiles of 512
PC = P // 128    # 16 pk chunks of 128

F32 = mybir.dt.float32
F16 = mybir.dt.float16
BF16 = mybir.dt.bfloat16
FP8 = mybir.dt.float8e4
NP_FP8 = ml_dtypes.float8_e4m3
NP_BF16 = ml_dtypes.bfloat16
EXP = mybir.ActivationFunctionType.Exp
DR = mybir.MatmulPerfMode.DoubleRow
MASKC = -240.0   # exp(0.125 * -240) = e^-30 -> exact fp8 zero

_BUILT = {}


def _emit(nc, tc, aps, ctx):
    outp = aps["outp"]      # [2048, 1024] f16

    consts = ctx.enter_context(tc.tile_pool(name="consts", bufs=1))
    xpool = ctx.enter_context(tc.tile_pool(name="xpool", bufs=2))
    wpool = ctx.enter_context(tc.tile_pool(name="wpool", bufs=1))
    qkpool = ctx.enter_context(tc.tile_pool(name="qkpool", bufs=4))
    vtpool = ctx.enter_context(tc.tile_pool(name="vtpool", bufs=4))
    vapool = ctx.enter_context(tc.tile_pool(name="vapool", bufs=4))
    expool = ctx.enter_context(tc.tile_pool(name="expool", bufs=8))
    zppool = ctx.enter_context(tc.tile_pool(name="zppool", bufs=4))
    rcpool = ctx.enter_context(tc.tile_pool(name="rcpool", bufs=2))
    bcpool = ctx.enter_context(tc.tile_pool(name="bcpool", bufs=2))
    opool = ctx.enter_context(tc.tile_pool(name="opool", bufs=3))

    # ---- constants (one DMA: eye | maskneg) ----
    eyemask = consts.tile([128, 384], BF16)
    nc.scalar.dma_start(eyemask[:], aps["eyemask"][:])
    eyebf = eyemask[:, 0:128]
    maskneg = eyemask[:, 128:384]
    bcol = consts.tile([128, 6], F32)   # (q0,k0,v0,q1,k1,v1)
    nc.scalar.dma_start(bcol[:], aps["bcol"][:])

    # ---- weights: three DMAs ----
    wqk8 = wpool.tile([128, 2, 2, MKD, 2, 128], FP8, tag="wqk", name="wqk8")
    nc.scalar.dma_start(wqk8[:], aps["wqk8"][:])
    wv16 = wpool.tile([128, 2, 8, 128], BF16, tag="wv", name="wv16")
    nc.scalar.dma_start(wv16[:], aps["wv16"][:])
    wsb = {}
    for pr in range(NPAIRS):
        wsb[("q", pr)] = wqk8[:, pr, 0]
        wsb[("k", pr)] = wqk8[:, pr, 1]
        wsb[("v", pr)] = wv16[:, pr]
    wosb = wpool.tile([128, 2, 1024], F16, tag="w", name="wo")
    nc.scalar.dma_start(wosb[:], aps["wo"][:])

    # ---- x: per seq-half tiles, one contiguous DMA each ----
    xsb = {}
    xbf = {}
    for h in (1, 0):   # h=1 halves feed the first three groups
        xt = xpool.tile([128, MKD, 2, 1024], FP8, tag="x", name=f"x8_{h}")
        nc.sync.dma_start(xt[:], aps["x8"][h])
        xsb[h] = xt
        xt = xpool.tile([128, 8, 1024], BF16, tag="xb", name=f"xb{h}")
        nc.sync.dma_start(xt[:], aps["xbf"][h])
        xbf[h] = xt

    qts, kts = {}, {}
    vas = {}
    BIDX = {"q": 0, "k": 1, "v": 2}

    # persistent vts staging tiles (ones row written once, reused)
    vts_tiles = []
    for i in range(4):
        vt = vtpool.tile([65, 512], BF16, tag="vt", name=f"vts{i}")
        nc.vector.memset(vt[64:65, :], 1.0)
        vts_tiles.append(vt)
    _vts_ctr = [0]

    for pr in range(NPAIRS):
        qt = qkpool.tile([128, 2048], F16, tag="qk", name=f"qT{pr}")
        kt = qkpool.tile([128, 2048], F16, tag="qk", name=f"kT{pr}")
        qts[pr], kts[pr] = qt, kt
        for h01 in range(2):
            va = vapool.tile([128, PC, 2, 80], FP8, tag="va",
                             name=f"va{pr}{h01}")
            vas[(pr, h01)] = va

    def proj_group(t, pr, h, ps_pool, pst_pool, seq=False):
        """One projection group: matmul of type t, pair pr, seq half h
        (pq columns [1024h, 1024h+1024)) into two [128,512] PSUM tiles
        (or one at a time when seq=True, for a 1-buffer ring); evacuate
        on DVE with the bias fold."""
        w = wsb[(t, pr)]

        def mm(pp, d):
            if t in ("q", "k"):
                for c in range(MKD):
                    nc.tensor.matmul(
                        pp[:],
                        w[:, c, :, :],
                        xsb[h][:, c, :, 512 * d:512 * (d + 1)],
                        start=(c == 0), stop=(c == MKD - 1),
                        perf_mode=DR,
                    )
            else:
                for mk in range(8):
                    nc.tensor.matmul(
                        pp[:],
                        w[:, mk, :],
                        xbf[h][:, mk, 512 * d:512 * (d + 1)],
                        start=(mk == 0), stop=(mk == 7),
                    )

        bias = bcol[:, BIDX[t] + 3 * pr:BIDX[t] + 3 * pr + 1]
        if t in ("q", "k"):
            dest = (qts if t == "q" else kts)[pr]
            for d in ((1, 0) if h == 1 else (0, 1)):
                pp = ps_pool.tile([128, 512], F32, tag="aux",
                                  name=f"prj_{t}{pr}{h}{d}")
                mm(pp, d)
                nc.vector.tensor_scalar_add(
                    dest[:, 1024 * h + 512 * d:1024 * h + 512 * (d + 1)],
                    pp[:], bias,
                )
        else:
            # v: per d-tile, evacuate both heads to vts staging, then
            # PE-transpose to [128 pk, 65] and cast into the fp8 va planes
            units = []
            for d in range(2):
                j4 = 2 * h + d
                pp = ps_pool.tile([128, 512], F32, tag="aux",
                                  name=f"prj_v{pr}{h}{d}")
                mm(pp, d)
                for h01 in range(2):
                    vt = vts_tiles[_vts_ctr[0] % 4]
                    _vts_ctr[0] += 1
                    nc.vector.tensor_scalar_add(
                        vt[0:64, :],
                        pp[64 * h01:64 * (h01 + 1), :],
                        bcol[64 * h01:64 * (h01 + 1),
                             BIDX[t] + 3 * pr:BIDX[t] + 3 * pr + 1],
                    )
                    units.append((vt, h01, j4))
            for vt, h01, j4 in units:
                pst = pst_pool.tile([128, 4, 66], BF16, tag="aux",
                                   name=f"pst{pr}{h01}{j4}")
                for c4 in range(4):
                    nc.tensor.transpose(
                        pst[:, c4, 0:65],
                        vt[:, 128 * c4:128 * (c4 + 1)],
                        eyebf[0:65, 0:65],
                    )
                vhi = vas[(pr, h01)][:, 4 * j4:4 * j4 + 4, 0, 0:65]
                nc.vector.tensor_copy(vhi, pst[:, :, 0:65])
                nc.vector.tensor_sub(
                    vas[(pr, h01)][:, 4 * j4:4 * j4 + 4, 1, 0:65],
                    pst[:, :, 0:65], vhi,
                )

    def attn_pair(pr, j, sps_pool, zps_pool, prev_finish=None):
        """Attention for head-pair pr, pq tile j: bf16 row-packed scores
        with PE-additive triangular mask, one exp per pk chunk (both
        heads), fp8 hi/lo-residual DoubleRow z accumulation, then
        broadcast-normalize into the fp16 zp tile (plane pr).

        The previous unit's z-drain + normalize (prev_finish) is emitted
        after this unit's first PRE score chunks so the in-order PE
        stream never stalls on the normalize chain; this unit's own
        drain is returned as a closure."""
        qt, kt = qts[pr], kts[pr]
        nchunk = PC - 4 * j
        PRE = min(3, nchunk)
        DW = min(4, nchunk)
        zpss = []
        descs = []
        state = {"zn": 0}

        def emit_z(zi):
            ex, i_, wp = descs[zi]
            for h01 in range(2):
                nc.tensor.matmul(
                    zpss[h01][:, 0:wp],
                    vas[(pr, h01)][:, i_, :, 0:65],
                    ex[:, 512 * h01:512 * h01 + wp]
                    .unsqueeze(1).broadcast_to((128, 2, wp)),
                    start=(zi == 0), stop=(zi == nchunk - 1),
                    perf_mode=DR,
                )
            state["zn"] = zi + 1

        def finish():
            for zi in range(state["zn"], nchunk):
                emit_z(zi)
            zpj = zp_tiles[j]
            for h01 in range(2):
                if j == PT - 1:
                    # fully-masked query row P-1: denom 0 -> 1
                    nc.vector.memset(zpss[h01][64:65, 511:512], 1.0)
                dsb = rcpool.tile([1, 512], F32, tag="dn")
                nc.vector.tensor_copy(dsb[:], zpss[h01][64:65, :])
                rcp = rcpool.tile([1, 512], F32, tag="rc")
                nc.vector.reciprocal_approx_fast(rcp[:], dsb[:])
                bc = bcpool.tile([64, 512], F32, tag="bc")
                nc.gpsimd.partition_broadcast(bc[:], rcp[:])
                nc.vector.tensor_mul(
                    zpj[64 * h01:64 * (h01 + 1), pr, :],
                    zpss[h01][0:64, :], bc[:],
                )

        for idx in range(nchunk):
            if idx == PRE:
                if prev_finish is not None:
                    prev_finish()
                for h01 in range(2):
                    zpss.append(zps_pool.tile(
                        [65, 512], F32, tag="zps",
                        name=f"zps{pr}{j}{h01}"))
            i_ = PC - 1 - idx              # descending pk chunks
            tt = i_ - 4 * j
            wp = min(512, 128 * (tt + 1))
            ex = expool.tile([128, 1024], FP8, tag="ex")
            sps = sps_pool.tile([128, 1024], F32, tag="sc")
            for h01 in range(2):
                rows = slice(64 * h01, 64 * (h01 + 1))
                nc.tensor.matmul(
                    sps[:, 512 * h01:512 * h01 + wp],
                    kt[rows, 128 * i_:128 * (i_ + 1)],
                    qt[rows, 512 * j:512 * j + wp],
                    start=True, stop=(tt >= 4),
                )
            if tt < 4:
                nc.tensor.matmul(
                    sps[:].rearrange(
                        "p (two f) -> p two f",
                        two=2)[:, :, 128 * tt:128 * tt + 128],
                    eyebf[:],
                    maskneg[:, 0:128]
                    .unsqueeze(1).broadcast_to((128, 2, 128)),
                    start=False, stop=True,
                    skip_group_check=True,
                )
            nc.scalar.activation(
                ex[:].rearrange(
                    "p (two f) -> p two f", two=2)[:, :, 0:wp],
                sps[:].rearrange(
                    "p (two f) -> p two f", two=2)[:, :, 0:wp],
                EXP, scale=0.125,
            )
            descs.append((ex, i_, wp))
            zi = idx - DW
            if zi >= 0 and idx >= PRE:
                emit_z(zi)
        return finish

    def outproj(j, ps_pool):
        """Output projection for pq tile j: fp8 DR over both pairs."""
        zpj = zp_tiles[j]
        for c4 in range(4):
            ck = 4 * j + c4
            osb = opool.tile([128, 1024], F16, tag="osb")
            pps = [ps_pool.tile([128, 512], F32, tag="aux",
                               name=f"op{ck}{mt}") for mt in range(2)]
            for pr in range(2):
                for mt in range(2):
                    nc.tensor.matmul(
                        pps[mt][:],
                        zpj[:, pr, 128 * c4:128 * (c4 + 1)],
                        wosb[:, pr, 512 * mt:512 * (mt + 1)],
                        start=(pr == 0), stop=(pr == 1),
                    )
            nc.scalar.copy(osb[:, 0:512], pps[0][:])
            nc.vector.tensor_copy(osb[:, 512:1024], pps[1][:])
            nc.sync.dma_start(outp[128 * ck:128 * (ck + 1), :], osb[:])

    zp_tiles = {}
    for j in range(PT):
        zp_tiles[j] = zppool.tile([128, 2, 512], F16, tag="zp",
                                  name=f"zp{j}")

    with tc.tile_pool(name="ps_m", bufs=2, space="PSUM") as ps_m, \
         tc.tile_pool(name="ps_zps", bufs=2, space="PSUM") as ps_zps, \
         tc.tile_pool(name="ps_aux", bufs=2, space="PSUM") as ps_aux:
        # h=1 halves first; j=3 units carry no inline z, so scores can
        # start before the v projection lands. Each unit's z-drain +
        # normalize (+ output projection) rides inside the next unit.
        def opfin(f, jj):
            def g():
                f()
                outproj(jj, ps_aux)
            return g

        # warm the PE p-state while the x DMA lands: dummy matmuls on
        # memset tiles (no DMA dependency)
        wz = consts.tile([128, 128], BF16, tag="warm", name="warmw")
        nc.vector.memset(wz[:], 0.0)
        wx = consts.tile([128, 512], BF16, tag="warm2", name="warmx")
        nc.vector.memset(wx[:], 0.0)
        for wi in range(12):
            wp_ = ps_m.tile([128, 512], F32, tag="sc", name=f"warm{wi}")
            nc.tensor.matmul(wp_[:], wz[:], wx[:], start=True, stop=True)
        proj_group("q", 0, 1, ps_aux, ps_aux)
        proj_group("k", 0, 1, ps_aux, ps_aux)
        fin = attn_pair(0, 3, ps_m, ps_zps)
        proj_group("v", 0, 1, ps_aux, ps_aux)
        fin = attn_pair(0, 2, ps_m, ps_zps, fin)
        proj_group("q", 0, 0, ps_aux, ps_aux)
        proj_group("k", 0, 0, ps_aux, ps_aux)
        proj_group("v", 0, 0, ps_aux, ps_aux)
        fin = attn_pair(0, 1, ps_m, ps_zps, fin)
        proj_group("q", 1, 1, ps_aux, ps_aux)
        proj_group("k", 1, 1, ps_aux, ps_aux)
        fin = attn_pair(0, 0, ps_m, ps_zps, fin)
        proj_group("v", 1, 1, ps_aux, ps_aux)
        fin = attn_pair(1, 3, ps_m, ps_zps, fin)
        proj_group("q", 1, 0, ps_aux, ps_aux)
        proj_group("k", 1, 0, ps_aux, ps_aux)
        fin = attn_pair(1, 2, ps_m, ps_zps, opfin(fin, 3))
        proj_group("v", 1, 0, ps_aux, ps_aux)
        fin = attn_pair(1, 1, ps_m, ps_zps, opfin(fin, 2))
        fin = attn_pair(1, 0, ps_m, ps_zps, opfin(fin, 1))
        fin()
        outproj(0, ps_aux)


def _build():
    if "v2" in _BUILT:
        return _BUILT["v2"]
    from contextlib import ExitStack

    nc = bacc.Bacc("TRN2", target_bir_lowering=False, debug=False)
    aps = {
        "x8": nc.dram_tensor("x8", [2, 128, MKD, 2, 1024], FP8,
                             kind="ExternalInput").ap(),
        "xbf": nc.dram_tensor("xbf", [2, 128, 8, 1024], BF16,
                              kind="ExternalInput").ap(),
        "wqk8": nc.dram_tensor("wqk8", [128, 2, 2, MKD, 2, 128], FP8,
                               kind="ExternalInput").ap(),
        "wv16": nc.dram_tensor("wv16", [128, 2, 8, 128], BF16,
                               kind="ExternalInput").ap(),
        "wo": nc.dram_tensor("wo", [128, 2, 1024], F16,
                             kind="ExternalInput").ap(),
        "bcol": nc.dram_tensor("bcol", [128, 6], F32,
                               kind="ExternalInput").ap(),
        "eyemask": nc.dram_tensor("eyemask", [128, 384], BF16,
                                  kind="ExternalInput").ap(),
        "outp": nc.dram_tensor("outp", [P, M], F16,
                               kind="ExternalOutput").ap(),
    }
    with tile.TileContext(nc) as tc:
        with ExitStack() as ctx, nc.allow_low_precision(
            reason="fp8 softmax kernel; verified numerically vs reference"
        ):
            _emit(nc, tc, aps, ctx)
    nc.compile()
    _BUILT["v2"] = nc
    return nc


def _host_inputs(x, kq, kk, kv, ko, bq, bk, bv):
    r = np.arange(128)
    m1 = np.where(r[None, :] >= r[:, None], MASKC, 0.0)  # block: mask c >= r
    eyemask = np.concatenate(
        [np.eye(128, dtype=np.float32), m1, np.full((128, 128), MASKC)],
        axis=1,
    ).astype(NP_BF16)  # [128, 384]

    in_maps = []
    for c in range(NCORES):
        b, k4 = divmod(c, 4)
        heads = [4 * k4 + i for i in range(HPC)]
        xT = np.ascontiguousarray(x[b].T)  # [1024, 2048]
        # x8[h][p, c, i, t'] = xT[256c + 128i + p, 1024h + t']
        x4 = xT.reshape(MKD, 2, 128, 2048).transpose(2, 0, 1, 3)
        x8 = np.ascontiguousarray(
            x4.reshape(128, MKD, 2, 2, 1024).transpose(3, 0, 1, 2, 4)
        ).astype(NP_FP8)  # [2, 128, MKD, 2, 1024]
        # xbf[h][p, mk, t'] = xT[128mk + p, 1024h + t']
        xb = xT.reshape(8, 128, 2, 1024).transpose(2, 1, 0, 3)
        xbf = np.ascontiguousarray(xb).astype(NP_BF16)  # [2, 128, 8, 1024]

        def pairm(kern, pr):
            return np.concatenate(
                [kern[heads[2 * pr]], kern[heads[2 * pr + 1]]], axis=1
            )  # [1024, 128]

        wqk8 = np.empty((128, 2, 2, MKD, 2, 128), NP_FP8)
        wv16 = np.empty((128, 2, 8, 128), NP_BF16)
        for pr in range(NPAIRS):
            for ti, kern in ((0, kq), (1, kk)):
                pm = pairm(kern, pr)  # [1024, 128]
                wqk8[:, pr, ti] = pm.reshape(MKD, 2, 128, 128).transpose(
                    2, 0, 1, 3).astype(NP_FP8)
            pmv = pairm(kv, pr)
            wv16[:, pr] = pmv.reshape(8, 128, 128).transpose(
                1, 0, 2).astype(NP_BF16)

        wo = np.stack(
            [np.concatenate([ko[heads[0]], ko[heads[1]]], axis=0),
             np.concatenate([ko[heads[2]], ko[heads[3]]], axis=0)], axis=1
        ).astype(np.float16)  # [128, 2, 1024]

        bcol = np.zeros((128, 6), np.float32)
        for pr in range(NPAIRS):
            for idx, bias in ((0, bq), (1, bk), (2, bv)):
                bcol[:, idx + 3 * pr] = np.concatenate(
                    [bias[heads[2 * pr]], bias[heads[2 * pr + 1]]]
                )

        in_maps.append({
            "x8": x8, "xbf": xbf,
            "wqk8": wqk8, "wv16": wv16,
            "wo": wo, "bcol": bcol,
            "eyemask": eyemask,
        })
    return in_maps


def kernel(x, kernel_query, kernel_key, kernel_value, kernel_out,
           bias_query, bias_key, bias_value, bias_out, _trace=False):
    x = np.asarray(x, np.float32)
    kq = np.asarray(kernel_query, np.float32)
    kk = np.asarray(kernel_key, np.float32)
    kv = np.asarray(kernel_value, np.float32)
    ko = np.asarray(kernel_out, np.float32)
    bq = np.asarray(bias_query, np.float32)
    bk = np.asarray(bias_key, np.float32)
    bv = np.asarray(bias_value, np.float32)
    bo = np.asarray(bias_out, np.float32)

    nc = _build()
    in_maps = _host_inputs(x, kq, kk, kv, ko, bq, bk, bv)
    res = bass_utils.run_bass_kernel_spmd(
        nc, in_maps, core_ids=list(range(NCORES)), trace=_trace
    )
    out = np.zeros((B, P, M), np.float32)
    for c in range(NCORES):
        out[c // 4] += res.results[c]["outp"].astype(np.float32)
    out += bo[None, None, :]

    # patch fully-masked query row P-1: uniform attention = mean_k v
    for b in range(B):
        xbar = x[b].mean(axis=0, dtype=np.float64)  # [M]
        row = np.zeros(M, np.float64)
        for n in range(N):
            zrow = xbar @ kv[n].astype(np.float64) + bv[n].astype(np.float64)
            row += zrow @ ko[n].astype(np.float64)
        out[b, P - 1, :] = (row + bo.astype(np.float64)).astype(np.float32)

    if _trace:
        kernel._last_result = res
    return out


# revision 42
# speedup vs baseline: 1.0058x; 1.0058x over previous
"""Trainium2 Bass kernel for multi-head attention (B=2, P=2048, M=1024, N=16, H=64).

out = softmax(mask(x@Wq @ (x@Wk)^T / sqrt(H))) @ (x@Wv) @ Wo + biases,
with the module's strictly-upper-triangular keep mask (row P-1 fully masked).

Sharding: 8 cores = 2 batches x 4 head-groups. Core c handles batch c//4,
heads [4*(c%4), 4*(c%4)+4); the host sums the 4 partial output projections
per batch and patches the fully-masked query row P-1 analytically.

v3 design (fp8 DoubleRow where precision allows, ACT-exp-bound):
  - q/k projections (K=1024) run as fp8e4 DoubleRow matmuls (2 K-chunks
    per pass, 0.5 cyc/row). The v projection runs in bf16: fp8 v errors
    hit concentrated-attention rows at full strength (measured).
  - z = v_aug @ exp keeps DoubleRow speed at 16-bit-grade v precision:
    v_aug^T is stored as an fp8 hi+lo residual pair (v = hi + lo + O(e^2))
    in the two DR planes, and the fp8 ex operand is duplicated across
    planes with a stride-0 AP. Output projection runs in fp16.
  - Scores stay bf16, two heads row-packed in disjoint PE row groups.
  - The triangular mask is applied ADDITIVELY on the PE (eye @ maskneg
    accumulated into score PSUM): exp then yields exact fp8 zeros, so no
    DVE masking and no garbage in the DoubleRow-widened columns.
  - Softmax denominators come from an appended ones column in v_aug^T;
    normalization = DVE reciprocal of the denom row -> gpsimd
    partition_broadcast -> one DVE multiply straight into the fp8 zp tile
    (replaces the baseline's 4 PE transposes per chunk).
  - ACT runs exp only (one activation per pk chunk covers both heads);
    all PSUM evacuation is on DVE; DMA issue on the sync queue.
  - x DMA is chunked/ordered so the first projection starts ~1us in.
  - Timeline: proj(pr0) -> attention(pr0, j=3..0) with proj(pr1)+v1
    transposes interleaved into PE slack -> attention(pr1) with the
    output projection per j trailing.
"""
import sys

import numpy as np

if "/opt/trn_rl_repo" not in sys.path:
    sys.path.insert(0, "/opt/trn_rl_repo")

import concourse.bacc as bacc
import concourse.tile as tile
from concourse import mybir
from concourse import bass_utils
import ml_dtypes

B, P, M, N, H = 2, 2048, 1024, 16, 64
NCORES = 8
HPC = 4          # heads per core
NPAIRS = 2       # head pairs per core
MKD = 4          # DoubleRow contraction chunks (256 each) for projections
PT = P // 512    # 4 pq tiles of 512
PC = P // 128    # 16 pk chunks of 128

F32 = mybir.dt.float32
F16 = mybir.dt.float16
BF16 = mybir.dt.bfloat16
FP8 = mybir.dt.float8e4
NP_FP8 = ml_dtypes.float8_e4m3
NP_BF16 = ml_dtypes.bfloat16
EXP = mybir.ActivationFunctionType.Exp
DR = mybir.MatmulPerfMode.DoubleRow
MASKC = -240.0   # exp(0.125 * -240) = e^-30 -> exact fp8 zero

_BUILT = {}


def _emit(nc, tc, aps, ctx):
    outp = aps["outp"]      # [2048, 1024] f16

    consts = ctx.enter_context(tc.tile_pool(name="consts", bufs=1))
    xpool = ctx.enter_context(tc.tile_pool(name="xpool", bufs=2))
    wpool = ctx.enter_context(tc.tile_pool(name="wpool", bufs=1))
    qkpool = ctx.enter_context(tc.tile_pool(name="qkpool", bufs=4))
    vtpool = ctx.enter_context(tc.tile_pool(name="vtpool", bufs=4))
    vapool = ctx.enter_context(tc.tile_pool(name="vapool", bufs=4))
    expool = ctx.enter_context(tc.tile_pool(name="expool", bufs=8))
    zppool = ctx.enter_context(tc.tile_pool(name="zppool", bufs=4))
    rcpool = ctx.enter_context(tc.tile_pool(name="rcpool", bufs=2))
    bcpool = ctx.enter_context(tc.tile_pool(name="bcpool", bufs=2))
    opool = ctx.enter_context(tc.tile_pool(name="opool", bufs=3))

    # ---- constants (one DMA: eye | maskneg) ----
    eyemask = consts.tile([128, 384], BF16)
    nc.scalar.dma_start(eyemask[:], aps["eyemask"][:])
    eyebf = eyemask[:, 0:128]
    maskneg = eyemask[:, 128:384]
    bcol = consts.tile([128, 6], F32)   # (q0,k0,v0,q1,k1,v1)
    nc.scalar.dma_start(bcol[:], aps["bcol"][:])

    # ---- weights: three DMAs ----
    wqk8 = wpool.tile([128, 2, 2, MKD, 2, 128], FP8, tag="wqk", name="wqk8")
    nc.scalar.dma_start(wqk8[:], aps["wqk8"][:])
    wv16 = wpool.tile([128, 2, 8, 128], BF16, tag="wv", name="wv16")
    nc.scalar.dma_start(wv16[:], aps["wv16"][:])
    wsb = {}
    for pr in range(NPAIRS):
        wsb[("q", pr)] = wqk8[:, pr, 0]
        wsb[("k", pr)] = wqk8[:, pr, 1]
        wsb[("v", pr)] = wv16[:, pr]
    wosb = wpool.tile([128, 2, 1024], F16, tag="w", name="wo")
    nc.scalar.dma_start(wosb[:], aps["wo"][:])

    # ---- x: per seq-half tiles, one contiguous DMA each ----
    xsb = {}
    xbf = {}
    for h in (1, 0):   # h=1 halves feed the first three groups
        xt = xpool.tile([128, MKD, 2, 1024], FP8, tag="x", name=f"x8_{h}")
        nc.sync.dma_start(xt[:], aps["x8"][h])
        xsb[h] = xt
        xt = xpool.tile([128, 8, 1024], BF16, tag="xb", name=f"xb{h}")
        nc.sync.dma_start(xt[:], aps["xbf"][h])
        xbf[h] = xt

    qts, kts = {}, {}
    vas = {}
    BIDX = {"q": 0, "k": 1, "v": 2}

    # persistent vts staging tiles (ones row written once, reused)
    vts_tiles = []
    for i in range(4):
        vt = vtpool.tile([65, 512], BF16, tag="vt", name=f"vts{i}")
        nc.vector.memset(vt[64:65, :], 1.0)
        vts_tiles.append(vt)
    _vts_ctr = [0]

    for pr in range(NPAIRS):
        qt = qkpool.tile([128, 2048], F16, tag="qk", name=f"qT{pr}")
        kt = qkpool.tile([128, 2048], F16, tag="qk", name=f"kT{pr}")
        qts[pr], kts[pr] = qt, kt
        for h01 in range(2):
            va = vapool.tile([128, PC, 2, 80], FP8, tag="va",
                             name=f"va{pr}{h01}")
            vas[(pr, h01)] = va

    def proj_group(t, pr, h, ps_pool, pst_pool, seq=False):
        """One projection group: matmul of type t, pair pr, seq half h
        (pq columns [1024h, 1024h+1024)) into two [128,512] PSUM tiles
        (or one at a time when seq=True, for a 1-buffer ring); evacuate
        on DVE with the bias fold."""
        w = wsb[(t, pr)]

        def mm(pp, d):
            if t in ("q", "k"):
                for c in range(MKD):
                    nc.tensor.matmul(
                        pp[:],
                        w[:, c, :, :],
                        xsb[h][:, c, :, 512 * d:512 * (d + 1)],
                        start=(c == 0), stop=(c == MKD - 1),
                        perf_mode=DR,
                    )
            else:
                for mk in range(8):
                    nc.tensor.matmul(
                        pp[:],
                        w[:, mk, :],
                        xbf[h][:, mk, 512 * d:512 * (d + 1)],
                        start=(mk == 0), stop=(mk == 7),
                    )

        bias = bcol[:, BIDX[t] + 3 * pr:BIDX[t] + 3 * pr + 1]
        if t in ("q", "k"):
            dest = (qts if t == "q" else kts)[pr]
            for d in ((1, 0) if h == 1 else (0, 1)):
                pp = ps_pool.tile([128, 512], F32, tag="aux",
                                  name=f"prj_{t}{pr}{h}{d}")
                mm(pp, d)
                nc.vector.tensor_scalar_add(
                    dest[:, 1024 * h + 512 * d:1024 * h + 512 * (d + 1)],
                    pp[:], bias,
                )
        else:
            # v: per d-tile, evacuate both heads to vts staging, then
            # PE-transpose to [128 pk, 65] and cast into the fp8 va planes
            units = []
            for d in range(2):
                j4 = 2 * h + d
                pp = ps_pool.tile([128, 512], F32, tag="aux",
                                  name=f"prj_v{pr}{h}{d}")
                mm(pp, d)
                for h01 in range(2):
                    vt = vts_tiles[_vts_ctr[0] % 4]
                    _vts_ctr[0] += 1
                    nc.vector.tensor_scalar_add(
                        vt[0:64, :],
                        pp[64 * h01:64 * (h01 + 1), :],
                        bcol[64 * h01:64 * (h01 + 1),
                             BIDX[t] + 3 * pr:BIDX[t] + 3 * pr + 1],
                    )
                    units.append((vt, h01, j4))
            for vt, h01, j4 in units:
                pst = pst_pool.tile([128, 4, 66], BF16, tag="aux",
                                   name=f"pst{pr}{h01}{j4}")
                for c4 in range(4):
                    nc.tensor.transpose(
                        pst[:, c4, 0:65],
                        vt[:, 128 * c4:128 * (c4 + 1)],
                        eyebf[0:65, 0:65],
                    )
                vhi = vas[(pr, h01)][:, 4 * j4:4 * j4 + 4, 0, 0:65]
                nc.vector.tensor_copy(vhi, pst[:, :, 0:65])
                nc.vector.tensor_sub(
                    vas[(pr, h01)][:, 4 * j4:4 * j4 + 4, 1, 0:65],
                    pst[:, :, 0:65], vhi,
                )

    def attn_pair(pr, j, sps_pool, zps_pool, prev_finish=None):
        """Attention for head-pair pr, pq tile j: bf16 row-packed scores
        with PE-additive triangular mask, one exp per pk chunk (both
        heads), fp8 hi/lo-residual DoubleRow z accumulation, then
        broadcast-normalize into the fp16 zp tile (plane pr).

        The previous unit's z-drain + normalize (prev_finish) is emitted
        after this unit's first PRE score chunks so the in-order PE
        stream never stalls on the normalize chain; this unit's own
        drain is returned as a closure."""
        qt, kt = qts[pr], kts[pr]
        nchunk = PC - 4 * j
        PRE = min(3, nchunk)
        DW = min(4, nchunk)
        zpss = []
        descs = []
        state = {"zn": 0}

        def emit_z(zi):
            ex, i_, wp = descs[zi]
            for h01 in range(2):
                nc.tensor.matmul(
                    zpss[h01][:, 0:wp],
                    vas[(pr, h01)][:, i_, :, 0:65],
                    ex[:, 512 * h01:512 * h01 + wp]
                    .unsqueeze(1).broadcast_to((128, 2, wp)),
                    start=(zi == 0), stop=(zi == nchunk - 1),
                    perf_mode=DR,
                )
            state["zn"] = zi + 1

        def finish():
            for zi in range(state["zn"], nchunk):
                emit_z(zi)
            zpj = zp_tiles[j]
            for h01 in range(2):
                if j == PT - 1:
                    # fully-masked query row P-1: denom 0 -> 1
                    nc.vector.memset(zpss[h01][64:65, 511:512], 1.0)
                dsb = rcpool.tile([1, 512], F32, tag="dn")
                nc.vector.tensor_copy(dsb[:], zpss[h01][64:65, :])
                rcp = rcpool.tile([1, 512], F32, tag="rc")
                nc.vector.reciprocal_approx_fast(rcp[:], dsb[:])
                bc = bcpool.tile([64, 512], F32, tag="bc")
                nc.gpsimd.partition_broadcast(bc[:], rcp[:])
                nc.vector.tensor_mul(
                    zpj[64 * h01:64 * (h01 + 1), pr, :],
                    zpss[h01][0:64, :], bc[:],
                )

        for idx in range(nchunk):
            if idx == PRE:
                if prev_finish is not None:
                    prev_finish()
                for h01 in range(2):
                    zpss.append(zps_pool.tile(
                        [65, 512], F32, tag="zps",
                        name=f"zps{pr}{j}{h01}"))
            i_ = PC - 1 - idx              # descending pk chunks
            tt = i_ - 4 * j
            wp = min(512, 128 * (tt + 1))
            ex = expool.tile([128, 1024], FP8, tag="ex")
            sps = sps_pool.tile([128, 1024], F32, tag="sc")
            for h01 in range(2):
                rows = slice(64 * h01, 64 * (h01 + 1))
                nc.tensor.matmul(
                    sps[:, 512 * h01:512 * h01 + wp],
                    kt[rows, 128 * i_:128 * (i_ + 1)],
                    qt[rows, 512 * j:512 * j + wp],
                    start=True, stop=(tt >= 4),
                )
            if tt < 4:
                nc.tensor.matmul(
                    sps[:].rearrange(
                        "p (two f) -> p two f",
                        two=2)[:, :, 128 * tt:128 * tt + 128],
                    eyebf[:],
                    maskneg[:, 0:128]
                    .unsqueeze(1).broadcast_to((128, 2, 128)),
                    start=False, stop=True,
                    skip_group_check=True,
                )
            nc.scalar.activation(
                ex[:].rearrange(
                    "p (two f) -> p two f", two=2)[:, :, 0:wp],
                sps[:].rearrange(
                    "p (two f) -> p two f", two=2)[:, :, 0:wp],
                EXP, scale=0.125,
            )
            descs.append((ex, i_, wp))
            zi = idx - DW
            if zi >= 0 and idx >= PRE:
                emit_z(zi)
        return finish

    def outproj(j, ps_pool):
        """Output projection for pq tile j: fp8 DR over both pairs."""
        zpj = zp_tiles[j]
        for c4 in range(4):
            ck = 4 * j + c4
            osb = opool.tile([128, 1024], F16, tag="osb")
            pps = [ps_pool.tile([128, 512], F32, tag="aux",
                               name=f"op{ck}{mt}") for mt in range(2)]
            for pr in range(2):
                for mt in range(2):
                    nc.tensor.matmul(
                        pps[mt][:],
                        zpj[:, pr, 128 * c4:128 * (c4 + 1)],
                        wosb[:, pr, 512 * mt:512 * (mt + 1)],
                        start=(pr == 0), stop=(pr == 1),
                    )
            for mt in range(2):
                nc.vector.tensor_copy(
                    osb[:, 512 * mt:512 * (mt + 1)], pps[mt][:])
            nc.sync.dma_start(outp[128 * ck:128 * (ck + 1), :], osb[:])

    zp_tiles = {}
    for j in range(PT):
        zp_tiles[j] = zppool.tile([128, 2, 512], F16, tag="zp",
                                  name=f"zp{j}")

    with tc.tile_pool(name="ps_m", bufs=2, space="PSUM") as ps_m, \
         tc.tile_pool(name="ps_zps", bufs=2, space="PSUM") as ps_zps, \
         tc.tile_pool(name="ps_aux", bufs=2, space="PSUM") as ps_aux:
        # h=1 halves first; j=3 units carry no inline z, so scores can
        # start before the v projection lands. Each unit's z-drain +
        # normalize (+ output projection) rides inside the next unit.
        def opfin(f, jj):
            def g():
                f()
                outproj(jj, ps_aux)
            return g

        # warm the PE p-state while the x DMA lands: dummy matmuls on
        # memset tiles (no DMA dependency)
        wz = consts.tile([128, 128], BF16, tag="warm", name="warmw")
        nc.vector.memset(wz[:], 0.0)
        wx = consts.tile([128, 512], BF16, tag="warm2", name="warmx")
        nc.vector.memset(wx[:], 0.0)
        for wi in range(12):
            wp_ = ps_m.tile([128, 512], F32, tag="sc", name=f"warm{wi}")
            nc.tensor.matmul(wp_[:], wz[:], wx[:], start=True, stop=True)
        proj_group("q", 0, 1, ps_aux, ps_aux)
        proj_group("k", 0, 1, ps_aux, ps_aux)
        fin = attn_pair(0, 3, ps_m, ps_zps)
        proj_group("v", 0, 1, ps_aux, ps_aux)
        fin = attn_pair(0, 2, ps_m, ps_zps, fin)
        proj_group("q", 0, 0, ps_aux, ps_aux)
        proj_group("k", 0, 0, ps_aux, ps_aux)
        proj_group("v", 0, 0, ps_aux, ps_aux)
        fin = attn_pair(0, 1, ps_m, ps_zps, fin)
        proj_group("q", 1, 1, ps_aux, ps_aux)
        proj_group("k", 1, 1, ps_aux, ps_aux)
        fin = attn_pair(0, 0, ps_m, ps_zps, fin)
        proj_group("v", 1, 1, ps_aux, ps_aux)
        fin = attn_pair(1, 3, ps_m, ps_zps, fin)
        proj_group("q", 1, 0, ps_aux, ps_aux)
        proj_group("k", 1, 0, ps_aux, ps_aux)
        fin = attn_pair(1, 2, ps_m, ps_zps, opfin(fin, 3))
        proj_group("v", 1, 0, ps_aux, ps_aux)
        fin = attn_pair(1, 1, ps_m, ps_zps, opfin(fin, 2))
        fin = attn_pair(1, 0, ps_m, ps_zps, opfin(fin, 1))
        fin()
        outproj(0, ps_aux)


def _build():
    if "v2" in _BUILT:
        return _BUILT["v2"]
    from contextlib import ExitStack

    nc = bacc.Bacc("TRN2", target_bir_lowering=False, debug=False)
    aps = {
        "x8": nc.dram_tensor("x8", [2, 128, MKD, 2, 1024], FP8,
                             kind="ExternalInput").ap(),
        "xbf": nc.dram_tensor("xbf", [2, 128, 8, 1024], BF16,
                              kind="ExternalInput").ap(),
        "wqk8": nc.dram_tensor("wqk8", [128, 2, 2, MKD, 2, 128], FP8,
                               kind="ExternalInput").ap(),
        "wv16": nc.dram_tensor("wv16", [128, 2, 8, 128], BF16,
                               kind="ExternalInput").ap(),
        "wo": nc.dram_tensor("wo", [128, 2, 1024], F16,
                             kind="ExternalInput").ap(),
        "bcol": nc.dram_tensor("bcol", [128, 6], F32,
                               kind="ExternalInput").ap(),
        "eyemask": nc.dram_tensor("eyemask", [128, 384], BF16,
                                  kind="ExternalInput").ap(),
        "outp": nc.dram_tensor("outp", [P, M], F16,
                               kind="ExternalOutput").ap(),
    }
    with tile.TileContext(nc) as tc:
        with ExitStack() as ctx, nc.allow_low_precision(
            reason="fp8 softmax kernel; verified numerically vs reference"
        ):
            _emit(nc, tc, aps, ctx)
    nc.compile()
    _BUILT["v2"] = nc
    return nc


def _host_inputs(x, kq, kk, kv, ko, bq, bk, bv):
    r = np.arange(128)
    m1 = np.where(r[None, :] >= r[:, None], MASKC, 0.0)  # block: mask c >= r
    eyemask = np.concatenate(
        [np.eye(128, dtype=np.float32), m1, np.full((128, 128), MASKC)],
        axis=1,
    ).astype(NP_BF16)  # [128, 384]

    in_maps = []
    for c in range(NCORES):
        b, k4 = divmod(c, 4)
        heads = [4 * k4 + i for i in range(HPC)]
        xT = np.ascontiguousarray(x[b].T)  # [1024, 2048]
        # x8[h][p, c, i, t'] = xT[256c + 128i + p, 1024h + t']
        x4 = xT.reshape(MKD, 2, 128, 2048).transpose(2, 0, 1, 3)
        x8 = np.ascontiguousarray(
            x4.reshape(128, MKD, 2, 2, 1024).transpose(3, 0, 1, 2, 4)
        ).astype(NP_FP8)  # [2, 128, MKD, 2, 1024]
        # xbf[h][p, mk, t'] = xT[128mk + p, 1024h + t']
        xb = xT.reshape(8, 128, 2, 1024).transpose(2, 1, 0, 3)
        xbf = np.ascontiguousarray(xb).astype(NP_BF16)  # [2, 128, 8, 1024]

        def pairm(kern, pr):
            return np.concatenate(
                [kern[heads[2 * pr]], kern[heads[2 * pr + 1]]], axis=1
            )  # [1024, 128]

        wqk8 = np.empty((128, 2, 2, MKD, 2, 128), NP_FP8)
        wv16 = np.empty((128, 2, 8, 128), NP_BF16)
        for pr in range(NPAIRS):
            for ti, kern in ((0, kq), (1, kk)):
                pm = pairm(kern, pr)  # [1024, 128]
                wqk8[:, pr, ti] = pm.reshape(MKD, 2, 128, 128).transpose(
                    2, 0, 1, 3).astype(NP_FP8)
            pmv = pairm(kv, pr)
            wv16[:, pr] = pmv.reshape(8, 128, 128).transpose(
                1, 0, 2).astype(NP_BF16)

        wo = np.stack(
            [np.concatenate([ko[heads[0]], ko[heads[1]]], axis=0),
             np.concatenate([ko[heads[2]], ko[heads[3]]], axis=0)], axis=1
        ).astype(np.float16)  # [128, 2, 1024]

        bcol = np.zeros((128, 6), np.float32)
        for pr in range(NPAIRS):
            for idx, bias in ((0, bq), (1, bk), (2, bv)):
                bcol[:, idx + 3 * pr] = np.concatenate(
                    [bias[heads[2 * pr]], bias[heads[2 * pr + 1]]]
                )

        in_maps.append({
            "x8": x8, "xbf": xbf,
            "wqk8": wqk8, "wv16": wv16,
            "wo": wo, "bcol": bcol,
            "eyemask": eyemask,
        })
    return in_maps


def kernel(x, kernel_query, kernel_key, kernel_value, kernel_out,
           bias_query, bias_key, bias_value, bias_out, _trace=False):
    x = np.asarray(x, np.float32)
    kq = np.asarray(kernel_query, np.float32)
    kk = np.asarray(kernel_key, np.float32)
    kv = np.asarray(kernel_value, np.float32)
    ko = np.asarray(kernel_out, np.float32)
    bq = np.asarray(bias_query, np.float32)
    bk = np.asarray(bias_key, np.float32)
    bv = np.asarray(bias_value, np.float32)
    bo = np.asarray(bias_out, np.float32)

    nc = _build()
    in_maps = _host_inputs(x, kq, kk, kv, ko, bq, bk, bv)
    res = bass_utils.run_bass_kernel_spmd(
        nc, in_maps, core_ids=list(range(NCORES)), trace=_trace
    )
    out = np.zeros((B, P, M), np.float32)
    for c in range(NCORES):
        out[c // 4] += res.results[c]["outp"].astype(np.float32)
    out += bo[None, None, :]

    # patch fully-masked query row P-1: uniform attention = mean_k v
    for b in range(B):
        xbar = x[b].mean(axis=0, dtype=np.float64)  # [M]
        row = np.zeros(M, np.float64)
        for n in range(N):
            zrow = xbar @ kv[n].astype(np.float64) + bv[n].astype(np.float64)
            row += zrow @ ko[n].astype(np.float64)
        out[b, P - 1, :] = (row + bo.astype(np.float64)).astype(np.float32)

    if _trace:
        kernel._last_result = res
    return out


# revision 43
# speedup vs baseline: 1.1739x; 1.1671x over previous
"""Trainium2 Bass kernel for multi-head attention (B=2, P=2048, M=1024, N=16, H=64).

out = softmax(mask(x@Wq @ (x@Wk)^T / sqrt(H))) @ (x@Wv) @ Wo + biases,
with the module's strictly-upper-triangular keep mask (row P-1 fully masked).

Sharding: 8 cores = 2 batches x 4 head-groups. Core c handles batch c//4,
heads [4*(c%4), 4*(c%4)+4); the host sums the 4 partial output projections
per batch and patches the fully-masked query row P-1 analytically.

v3 design (fp8 DoubleRow where precision allows, ACT-exp-bound):
  - q/k projections (K=1024) run as fp8e4 DoubleRow matmuls (2 K-chunks
    per pass, 0.5 cyc/row). The v projection runs in bf16: fp8 v errors
    hit concentrated-attention rows at full strength (measured).
  - z = v_aug @ exp keeps DoubleRow speed at 16-bit-grade v precision:
    v_aug^T is stored as an fp8 hi+lo residual pair (v = hi + lo + O(e^2))
    in the two DR planes, and the fp8 ex operand is duplicated across
    planes with a stride-0 AP. Output projection runs in fp16.
  - Scores stay bf16, two heads row-packed in disjoint PE row groups.
  - The triangular mask is applied ADDITIVELY on the PE (eye @ maskneg
    accumulated into score PSUM): exp then yields exact fp8 zeros, so no
    DVE masking and no garbage in the DoubleRow-widened columns.
  - Softmax denominators come from an appended ones column in v_aug^T;
    normalization = DVE reciprocal of the denom row -> gpsimd
    partition_broadcast -> one DVE multiply straight into the fp8 zp tile
    (replaces the baseline's 4 PE transposes per chunk).
  - ACT runs exp only (one activation per pk chunk covers both heads);
    all PSUM evacuation is on DVE; DMA issue on the sync queue.
  - x DMA is chunked/ordered so the first projection starts ~1us in.
  - Timeline: proj(pr0) -> attention(pr0, j=3..0) with proj(pr1)+v1
    transposes interleaved into PE slack -> attention(pr1) with the
    output projection per j trailing.
"""
import sys

import numpy as np

if "/opt/trn_rl_repo" not in sys.path:
    sys.path.insert(0, "/opt/trn_rl_repo")

import concourse.bacc as bacc
import concourse.tile as tile
from concourse import mybir
from concourse import bass_utils
import ml_dtypes

B, P, M, N, H = 2, 2048, 1024, 16, 64
NCORES = 8
HPC = 4          # heads per core
NPAIRS = 2       # head pairs per core
MKD = 4          # DoubleRow contraction chunks (256 each) for projections
PT = P // 512    # 4 pq tiles of 512
PC = P // 128    # 16 pk chunks of 128

F32 = mybir.dt.float32
F16 = mybir.dt.float16
BF16 = mybir.dt.bfloat16
FP8 = mybir.dt.float8e4
NP_FP8 = ml_dtypes.float8_e4m3
NP_BF16 = ml_dtypes.bfloat16
EXP = mybir.ActivationFunctionType.Exp
DR = mybir.MatmulPerfMode.DoubleRow
MASKC = -240.0   # exp(0.125 * -240) = e^-30 -> exact fp8 zero

_BUILT = {}


def _emit(nc, tc, aps, ctx):
    outp = aps["outp"]      # [2048, 1024] f16

    consts = ctx.enter_context(tc.tile_pool(name="consts", bufs=1))
    xpool = ctx.enter_context(tc.tile_pool(name="xpool", bufs=2))
    wpool = ctx.enter_context(tc.tile_pool(name="wpool", bufs=1))
    qkpool = ctx.enter_context(tc.tile_pool(name="qkpool", bufs=4))
    vtpool = ctx.enter_context(tc.tile_pool(name="vtpool", bufs=4))
    vapool = ctx.enter_context(tc.tile_pool(name="vapool", bufs=4))
    expool = ctx.enter_context(tc.tile_pool(name="expool", bufs=8))
    zppool = ctx.enter_context(tc.tile_pool(name="zppool", bufs=4))
    rcpool = ctx.enter_context(tc.tile_pool(name="rcpool", bufs=2))
    bcpool = ctx.enter_context(tc.tile_pool(name="bcpool", bufs=2))
    opool = ctx.enter_context(tc.tile_pool(name="opool", bufs=3))

    # ---- constants (one DMA: eye | maskneg) ----
    eyemask = consts.tile([128, 384], BF16)
    nc.scalar.dma_start(eyemask[:], aps["eyemask"][:])
    eyebf = eyemask[:, 0:128]
    maskneg = eyemask[:, 128:384]
    bcol = consts.tile([128, 6], F32)   # (q0,k0,v0,q1,k1,v1)
    nc.scalar.dma_start(bcol[:], aps["bcol"][:])

    # ---- weights: three DMAs ----
    wqk8 = wpool.tile([128, 2, 2, MKD, 2, 128], FP8, tag="wqk", name="wqk8")
    nc.scalar.dma_start(wqk8[:], aps["wqk8"][:])
    wv16 = wpool.tile([128, 2, 8, 128], BF16, tag="wv", name="wv16")
    nc.scalar.dma_start(wv16[:], aps["wv16"][:])
    wsb = {}
    for pr in range(NPAIRS):
        wsb[("q", pr)] = wqk8[:, pr, 0]
        wsb[("k", pr)] = wqk8[:, pr, 1]
        wsb[("v", pr)] = wv16[:, pr]
    wosb = wpool.tile([128, 2, 1024], F16, tag="w", name="wo")
    nc.scalar.dma_start(wosb[:], aps["wo"][:])

    # ---- x: per seq-half tiles, one contiguous DMA each ----
    xsb = {}
    xbf = {}
    for h in (1, 0):   # h=1 halves feed the first three groups
        xt = xpool.tile([128, MKD, 2, 1024], FP8, tag="x", name=f"x8_{h}")
        nc.sync.dma_start(xt[:], aps["x8"][h])
        xsb[h] = xt
        xt = xpool.tile([128, 8, 1024], BF16, tag="xb", name=f"xb{h}")
        nc.sync.dma_start(xt[:], aps["xbf"][h])
        xbf[h] = xt

    qts, kts = {}, {}
    vas = {}
    BIDX = {"q": 0, "k": 1, "v": 2}

    # persistent vts staging tiles (ones row written once, reused)
    vts_tiles = []
    for i in range(4):
        vt = vtpool.tile([65, 512], BF16, tag="vt", name=f"vts{i}")
        nc.vector.memset(vt[64:65, :], 1.0)
        vts_tiles.append(vt)
    _vts_ctr = [0]

    for pr in range(NPAIRS):
        qt = qkpool.tile([128, 2048], F16, tag="qk", name=f"qT{pr}")
        kt = qkpool.tile([128, 2048], F16, tag="qk", name=f"kT{pr}")
        qts[pr], kts[pr] = qt, kt
        for h01 in range(2):
            va = vapool.tile([128, PC, 2, 80], FP8, tag="va",
                             name=f"va{pr}{h01}")
            vas[(pr, h01)] = va

    def proj_group(t, pr, h, ps_pool, pst_pool, seq=False):
        """One projection group: matmul of type t, pair pr, seq half h
        (pq columns [1024h, 1024h+1024)) into two [128,512] PSUM tiles
        (or one at a time when seq=True, for a 1-buffer ring); evacuate
        on DVE with the bias fold."""
        w = wsb[(t, pr)]

        def mm(pp, d):
            if t in ("q", "k"):
                for c in range(MKD):
                    nc.tensor.matmul(
                        pp[:],
                        w[:, c, :, :],
                        xsb[h][:, c, :, 512 * d:512 * (d + 1)],
                        start=(c == 0), stop=(c == MKD - 1),
                        perf_mode=DR,
                    )
            else:
                for mk in range(8):
                    nc.tensor.matmul(
                        pp[:],
                        w[:, mk, :],
                        xbf[h][:, mk, 512 * d:512 * (d + 1)],
                        start=(mk == 0), stop=(mk == 7),
                    )

        bias = bcol[:, BIDX[t] + 3 * pr:BIDX[t] + 3 * pr + 1]
        if t in ("q", "k"):
            dest = (qts if t == "q" else kts)[pr]
            for d in ((1, 0) if h == 1 else (0, 1)):
                pp = ps_pool.tile([128, 512], F32, tag="aux",
                                  name=f"prj_{t}{pr}{h}{d}")
                mm(pp, d)
                nc.vector.tensor_scalar_add(
                    dest[:, 1024 * h + 512 * d:1024 * h + 512 * (d + 1)],
                    pp[:], bias,
                )
        else:
            # v: per d-tile, evacuate both heads to vts staging, then
            # PE-transpose to [128 pk, 65] and cast into the fp8 va planes
            units = []
            for d in range(2):
                j4 = 2 * h + d
                pp = ps_pool.tile([128, 512], F32, tag="aux",
                                  name=f"prj_v{pr}{h}{d}")
                mm(pp, d)
                for h01 in range(2):
                    vt = vts_tiles[_vts_ctr[0] % 4]
                    _vts_ctr[0] += 1
                    nc.vector.tensor_scalar_add(
                        vt[0:64, :],
                        pp[64 * h01:64 * (h01 + 1), :],
                        bcol[64 * h01:64 * (h01 + 1),
                             BIDX[t] + 3 * pr:BIDX[t] + 3 * pr + 1],
                    )
                    units.append((vt, h01, j4))
            for vt, h01, j4 in units:
                pst = pst_pool.tile([128, 4, 66], BF16, tag="aux",
                                   name=f"pst{pr}{h01}{j4}")
                for c4 in range(4):
                    nc.tensor.transpose(
                        pst[:, c4, 0:65],
                        vt[:, 128 * c4:128 * (c4 + 1)],
                        eyebf[0:65, 0:65],
                    )
                vhi = vas[(pr, h01)][:, 4 * j4:4 * j4 + 4, 0, 0:65]
                nc.vector.tensor_copy(vhi, pst[:, :, 0:65])
                nc.vector.tensor_sub(
                    vas[(pr, h01)][:, 4 * j4:4 * j4 + 4, 1, 0:65],
                    pst[:, :, 0:65], vhi,
                )

    def attn_pair(pr, j, sps_pool, zps_pool, prev_finish=None):
        """Attention for head-pair pr, pq tile j: bf16 row-packed scores
        with PE-additive triangular mask, one exp per pk chunk (both
        heads), fp8 hi/lo-residual DoubleRow z accumulation, then
        broadcast-normalize into the fp16 zp tile (plane pr).

        The previous unit's z-drain + normalize (prev_finish) is emitted
        after this unit's first PRE score chunks so the in-order PE
        stream never stalls on the normalize chain; this unit's own
        drain is returned as a closure."""
        qt, kt = qts[pr], kts[pr]
        nchunk = PC - 4 * j
        PRE = min(2, nchunk)
        DW = min(4, nchunk)
        zpss = []
        descs = []
        state = {"zn": 0}

        def emit_z(zi):
            ex, i_, wp = descs[zi]
            for h01 in range(2):
                nc.tensor.matmul(
                    zpss[h01][:, 0:wp],
                    vas[(pr, h01)][:, i_, :, 0:65],
                    ex[:, 512 * h01:512 * h01 + wp]
                    .unsqueeze(1).broadcast_to((128, 2, wp)),
                    start=(zi == 0), stop=(zi == nchunk - 1),
                    perf_mode=DR,
                )
            state["zn"] = zi + 1

        def finish():
            for zi in range(state["zn"], nchunk):
                emit_z(zi)
            zpj = zp_tiles[j]
            for h01 in range(2):
                if j == PT - 1:
                    # fully-masked query row P-1: denom 0 -> 1
                    nc.vector.memset(zpss[h01][64:65, 511:512], 1.0)
                dsb = rcpool.tile([1, 512], F32, tag="dn")
                nc.vector.tensor_copy(dsb[:], zpss[h01][64:65, :])
                rcp = rcpool.tile([1, 512], F32, tag="rc")
                nc.vector.reciprocal_approx_fast(rcp[:], dsb[:])
                bc = bcpool.tile([64, 512], F32, tag="bc")
                nc.gpsimd.partition_broadcast(bc[:], rcp[:])
                nc.vector.tensor_mul(
                    zpj[64 * h01:64 * (h01 + 1), pr, :],
                    zpss[h01][0:64, :], bc[:],
                )

        for idx in range(nchunk):
            if idx == PRE:
                if prev_finish is not None:
                    prev_finish()
                for h01 in range(2):
                    zpss.append(zps_pool.tile(
                        [65, 512], F32, tag="zps",
                        name=f"zps{pr}{j}{h01}"))
            i_ = PC - 1 - idx              # descending pk chunks
            tt = i_ - 4 * j
            wp = min(512, 128 * (tt + 1))
            ex = expool.tile([128, 1024], FP8, tag="ex")
            sps = sps_pool.tile([128, 1024], F32, tag="sc")
            for h01 in range(2):
                rows = slice(64 * h01, 64 * (h01 + 1))
                nc.tensor.matmul(
                    sps[:, 512 * h01:512 * h01 + wp],
                    kt[rows, 128 * i_:128 * (i_ + 1)],
                    qt[rows, 512 * j:512 * j + wp],
                    start=True, stop=(tt >= 4),
                )
            if tt < 4:
                nc.tensor.matmul(
                    sps[:].rearrange(
                        "p (two f) -> p two f",
                        two=2)[:, :, 128 * tt:128 * tt + 128],
                    eyebf[:],
                    maskneg[:, 0:128]
                    .unsqueeze(1).broadcast_to((128, 2, 128)),
                    start=False, stop=True,
                    skip_group_check=True,
                )
            nc.scalar.activation(
                ex[:].rearrange(
                    "p (two f) -> p two f", two=2)[:, :, 0:wp],
                sps[:].rearrange(
                    "p (two f) -> p two f", two=2)[:, :, 0:wp],
                EXP, scale=0.125,
            )
            descs.append((ex, i_, wp))
            zi = idx - DW
            if zi >= 0 and idx >= PRE:
                emit_z(zi)
        return finish

    def outproj(j, ps_pool):
        """Output projection for pq tile j: fp8 DR over both pairs."""
        zpj = zp_tiles[j]
        for c4 in range(4):
            ck = 4 * j + c4
            osb = opool.tile([128, 1024], F16, tag="osb")
            pps = [ps_pool.tile([128, 512], F32, tag="aux",
                               name=f"op{ck}{mt}") for mt in range(2)]
            for pr in range(2):
                for mt in range(2):
                    nc.tensor.matmul(
                        pps[mt][:],
                        zpj[:, pr, 128 * c4:128 * (c4 + 1)],
                        wosb[:, pr, 512 * mt:512 * (mt + 1)],
                        start=(pr == 0), stop=(pr == 1),
                    )
            for mt in range(2):
                nc.vector.tensor_copy(
                    osb[:, 512 * mt:512 * (mt + 1)], pps[mt][:])
            nc.sync.dma_start(outp[128 * ck:128 * (ck + 1), :], osb[:])

    zp_tiles = {}
    for j in range(PT):
        zp_tiles[j] = zppool.tile([128, 2, 512], F16, tag="zp",
                                  name=f"zp{j}")

    with tc.tile_pool(name="ps_m", bufs=2, space="PSUM") as ps_m, \
         tc.tile_pool(name="ps_zps", bufs=2, space="PSUM") as ps_zps, \
         tc.tile_pool(name="ps_aux", bufs=2, space="PSUM") as ps_aux:
        # h=1 halves first; j=3 units carry no inline z, so scores can
        # start before the v projection lands. Each unit's z-drain +
        # normalize (+ output projection) rides inside the next unit.
        def opfin(f, jj):
            def g():
                f()
                outproj(jj, ps_aux)
            return g

        # warm the PE p-state while the x DMA lands: dummy matmuls on
        # memset tiles (no DMA dependency)
        wz = consts.tile([128, 128], BF16, tag="warm", name="warmw")
        nc.vector.memset(wz[:], 0.0)
        wx = consts.tile([128, 512], BF16, tag="warm2", name="warmx")
        nc.vector.memset(wx[:], 0.0)
        for wi in range(12):
            wp_ = ps_m.tile([128, 512], F32, tag="sc", name=f"warm{wi}")
            nc.tensor.matmul(wp_[:], wz[:], wx[:], start=True, stop=True)
        proj_group("q", 0, 1, ps_aux, ps_aux)
        proj_group("k", 0, 1, ps_aux, ps_aux)
        fin = attn_pair(0, 3, ps_m, ps_zps)
        proj_group("v", 0, 1, ps_aux, ps_aux)
        fin = attn_pair(0, 2, ps_m, ps_zps, fin)
        proj_group("q", 0, 0, ps_aux, ps_aux)
        proj_group("k", 0, 0, ps_aux, ps_aux)
        proj_group("v", 0, 0, ps_aux, ps_aux)
        fin = attn_pair(0, 1, ps_m, ps_zps, fin)
        proj_group("q", 1, 1, ps_aux, ps_aux)
        proj_group("k", 1, 1, ps_aux, ps_aux)
        fin = attn_pair(0, 0, ps_m, ps_zps, fin)
        proj_group("v", 1, 1, ps_aux, ps_aux)
        fin = attn_pair(1, 3, ps_m, ps_zps, fin)
        proj_group("q", 1, 0, ps_aux, ps_aux)
        proj_group("k", 1, 0, ps_aux, ps_aux)
        fin = attn_pair(1, 2, ps_m, ps_zps, opfin(fin, 3))
        proj_group("v", 1, 0, ps_aux, ps_aux)
        fin = attn_pair(1, 1, ps_m, ps_zps, opfin(fin, 2))
        fin = attn_pair(1, 0, ps_m, ps_zps, opfin(fin, 1))
        fin()
        outproj(0, ps_aux)


def _build():
    if "v2" in _BUILT:
        return _BUILT["v2"]
    from contextlib import ExitStack

    nc = bacc.Bacc("TRN2", target_bir_lowering=False, debug=False)
    aps = {
        "x8": nc.dram_tensor("x8", [2, 128, MKD, 2, 1024], FP8,
                             kind="ExternalInput").ap(),
        "xbf": nc.dram_tensor("xbf", [2, 128, 8, 1024], BF16,
                              kind="ExternalInput").ap(),
        "wqk8": nc.dram_tensor("wqk8", [128, 2, 2, MKD, 2, 128], FP8,
                               kind="ExternalInput").ap(),
        "wv16": nc.dram_tensor("wv16", [128, 2, 8, 128], BF16,
                               kind="ExternalInput").ap(),
        "wo": nc.dram_tensor("wo", [128, 2, 1024], F16,
                             kind="ExternalInput").ap(),
        "bcol": nc.dram_tensor("bcol", [128, 6], F32,
                               kind="ExternalInput").ap(),
        "eyemask": nc.dram_tensor("eyemask", [128, 384], BF16,
                                  kind="ExternalInput").ap(),
        "outp": nc.dram_tensor("outp", [P, M], F16,
                               kind="ExternalOutput").ap(),
    }
    with tile.TileContext(nc) as tc:
        with ExitStack() as ctx, nc.allow_low_precision(
            reason="fp8 softmax kernel; verified numerically vs reference"
        ):
            _emit(nc, tc, aps, ctx)
    nc.compile()
    _BUILT["v2"] = nc
    return nc


def _host_inputs(x, kq, kk, kv, ko, bq, bk, bv):
    r = np.arange(128)
    m1 = np.where(r[None, :] >= r[:, None], MASKC, 0.0)  # block: mask c >= r
    eyemask = np.concatenate(
        [np.eye(128, dtype=np.float32), m1, np.full((128, 128), MASKC)],
        axis=1,
    ).astype(NP_BF16)  # [128, 384]

    in_maps = []
    for c in range(NCORES):
        b, k4 = divmod(c, 4)
        heads = [4 * k4 + i for i in range(HPC)]
        xT = np.ascontiguousarray(x[b].T)  # [1024, 2048]
        # x8[h][p, c, i, t'] = xT[256c + 128i + p, 1024h + t']
        x4 = xT.reshape(MKD, 2, 128, 2048).transpose(2, 0, 1, 3)
        x8 = np.ascontiguousarray(
            x4.reshape(128, MKD, 2, 2, 1024).transpose(3, 0, 1, 2, 4)
        ).astype(NP_FP8)  # [2, 128, MKD, 2, 1024]
        # xbf[h][p, mk, t'] = xT[128mk + p, 1024h + t']
        xb = xT.reshape(8, 128, 2, 1024).transpose(2, 1, 0, 3)
        xbf = np.ascontiguousarray(xb).astype(NP_BF16)  # [2, 128, 8, 1024]

        def pairm(kern, pr):
            return np.concatenate(
                [kern[heads[2 * pr]], kern[heads[2 * pr + 1]]], axis=1
            )  # [1024, 128]

        wqk8 = np.empty((128, 2, 2, MKD, 2, 128), NP_FP8)
        wv16 = np.empty((128, 2, 8, 128), NP_BF16)
        for pr in range(NPAIRS):
            for ti, kern in ((0, kq), (1, kk)):
                pm = pairm(kern, pr)  # [1024, 128]
                wqk8[:, pr, ti] = pm.reshape(MKD, 2, 128, 128).transpose(
                    2, 0, 1, 3).astype(NP_FP8)
            pmv = pairm(kv, pr)
            wv16[:, pr] = pmv.reshape(8, 128, 128).transpose(
                1, 0, 2).astype(NP_BF16)

        wo = np.stack(
            [np.concatenate([ko[heads[0]], ko[heads[1]]], axis=0),
             np.concatenate([ko[heads[2]], ko[heads[3]]], axis=0)], axis=1
        ).astype(np.float16)  # [128, 2, 1024]

        bcol = np.zeros((128, 6), np.float32)
        for pr in range(NPAIRS):
            for idx, bias in ((0, bq), (1, bk), (2, bv)):
                bcol[:, idx + 3 * pr] = np.concatenate(
                    [bias[heads[2 * pr]], bias[heads[2 * pr + 1]]]
                )

        in_maps.append({
            "x8": x8, "xbf": xbf,
            "wqk8": wqk8, "wv16": wv16,
            "wo": wo, "bcol": bcol,
            "eyemask": eyemask,
        })
    return in_maps


def kernel(x, kernel_query, kernel_key, kernel_value, kernel_out,
           bias_query, bias_key, bias_value, bias_out, _trace=False):
    x = np.asarray(x, np.float32)
    kq = np.asarray(kernel_query, np.float32)
    kk = np.asarray(kernel_key, np.float32)
    kv = np.asarray(kernel_value, np.float32)
    ko = np.asarray(kernel_out, np.float32)
    bq = np.asarray(bias_query, np.float32)
    bk = np.asarray(bias_key, np.float32)
    bv = np.asarray(bias_value, np.float32)
    bo = np.asarray(bias_out, np.float32)

    nc = _build()
    in_maps = _host_inputs(x, kq, kk, kv, ko, bq, bk, bv)
    res = bass_utils.run_bass_kernel_spmd(
        nc, in_maps, core_ids=list(range(NCORES)), trace=_trace
    )
    out = np.zeros((B, P, M), np.float32)
    for c in range(NCORES):
        out[c // 4] += res.results[c]["outp"].astype(np.float32)
    out += bo[None, None, :]

    # patch fully-masked query row P-1: uniform attention = mean_k v
    for b in range(B):
        xbar = x[b].mean(axis=0, dtype=np.float64)  # [M]
        row = np.zeros(M, np.float64)
        for n in range(N):
            zrow = xbar @ kv[n].astype(np.float64) + bv[n].astype(np.float64)
            row += zrow @ ko[n].astype(np.float64)
        out[b, P - 1, :] = (row + bo.astype(np.float64)).astype(np.float32)

    if _trace:
        kernel._last_result = res
    return out


# revision 44
# speedup vs baseline: 1.1974x; 1.0201x over previous
"""Trainium2 Bass kernel for multi-head attention (B=2, P=2048, M=1024, N=16, H=64).

out = softmax(mask(x@Wq @ (x@Wk)^T / sqrt(H))) @ (x@Wv) @ Wo + biases,
with the module's strictly-upper-triangular keep mask (row P-1 fully masked).

Sharding: 8 cores = 2 batches x 4 head-groups. Core c handles batch c//4,
heads [4*(c%4), 4*(c%4)+4); the host sums the 4 partial output projections
per batch and patches the fully-masked query row P-1 analytically.

v3 design (fp8 DoubleRow where precision allows, ACT-exp-bound):
  - q/k projections (K=1024) run as fp8e4 DoubleRow matmuls (2 K-chunks
    per pass, 0.5 cyc/row). The v projection runs in bf16: fp8 v errors
    hit concentrated-attention rows at full strength (measured).
  - z = v_aug @ exp keeps DoubleRow speed at 16-bit-grade v precision:
    v_aug^T is stored as an fp8 hi+lo residual pair (v = hi + lo + O(e^2))
    in the two DR planes, and the fp8 ex operand is duplicated across
    planes with a stride-0 AP. Output projection runs in fp16.
  - Scores stay bf16, two heads row-packed in disjoint PE row groups.
  - The triangular mask is applied ADDITIVELY on the PE (eye @ maskneg
    accumulated into score PSUM): exp then yields exact fp8 zeros, so no
    DVE masking and no garbage in the DoubleRow-widened columns.
  - Softmax denominators come from an appended ones column in v_aug^T;
    normalization = DVE reciprocal of the denom row -> gpsimd
    partition_broadcast -> one DVE multiply straight into the fp8 zp tile
    (replaces the baseline's 4 PE transposes per chunk).
  - ACT runs exp only (one activation per pk chunk covers both heads);
    all PSUM evacuation is on DVE; DMA issue on the sync queue.
  - x DMA is chunked/ordered so the first projection starts ~1us in.
  - Timeline: proj(pr0) -> attention(pr0, j=3..0) with proj(pr1)+v1
    transposes interleaved into PE slack -> attention(pr1) with the
    output projection per j trailing.
"""
import sys

import numpy as np

if "/opt/trn_rl_repo" not in sys.path:
    sys.path.insert(0, "/opt/trn_rl_repo")

import concourse.bacc as bacc
import concourse.tile as tile
from concourse import mybir
from concourse import bass_utils
import ml_dtypes

B, P, M, N, H = 2, 2048, 1024, 16, 64
NCORES = 8
HPC = 4          # heads per core
NPAIRS = 2       # head pairs per core
MKD = 4          # DoubleRow contraction chunks (256 each) for projections
PT = P // 512    # 4 pq tiles of 512
PC = P // 128    # 16 pk chunks of 128

F32 = mybir.dt.float32
F16 = mybir.dt.float16
BF16 = mybir.dt.bfloat16
FP8 = mybir.dt.float8e4
NP_FP8 = ml_dtypes.float8_e4m3
NP_BF16 = ml_dtypes.bfloat16
EXP = mybir.ActivationFunctionType.Exp
DR = mybir.MatmulPerfMode.DoubleRow
MASKC = -240.0   # exp(0.125 * -240) = e^-30 -> exact fp8 zero

_BUILT = {}


def _emit(nc, tc, aps, ctx):
    outp = aps["outp"]      # [2048, 1024] f16

    consts = ctx.enter_context(tc.tile_pool(name="consts", bufs=1))
    xpool = ctx.enter_context(tc.tile_pool(name="xpool", bufs=2))
    wpool = ctx.enter_context(tc.tile_pool(name="wpool", bufs=1))
    qkpool = ctx.enter_context(tc.tile_pool(name="qkpool", bufs=4))
    vtpool = ctx.enter_context(tc.tile_pool(name="vtpool", bufs=4))
    vapool = ctx.enter_context(tc.tile_pool(name="vapool", bufs=4))
    expool = ctx.enter_context(tc.tile_pool(name="expool", bufs=10))
    zppool = ctx.enter_context(tc.tile_pool(name="zppool", bufs=4))
    rcpool = ctx.enter_context(tc.tile_pool(name="rcpool", bufs=2))
    bcpool = ctx.enter_context(tc.tile_pool(name="bcpool", bufs=2))
    opool = ctx.enter_context(tc.tile_pool(name="opool", bufs=3))

    # ---- constants (one DMA: eye | maskneg) ----
    eyemask = consts.tile([128, 384], BF16)
    nc.scalar.dma_start(eyemask[:], aps["eyemask"][:])
    eyebf = eyemask[:, 0:128]
    maskneg = eyemask[:, 128:384]
    bcol = consts.tile([128, 6], F32)   # (q0,k0,v0,q1,k1,v1)
    nc.scalar.dma_start(bcol[:], aps["bcol"][:])

    # ---- weights: three DMAs ----
    wqk8 = wpool.tile([128, 2, 2, MKD, 2, 128], FP8, tag="wqk", name="wqk8")
    nc.scalar.dma_start(wqk8[:], aps["wqk8"][:])
    wv16 = wpool.tile([128, 2, 8, 128], BF16, tag="wv", name="wv16")
    nc.scalar.dma_start(wv16[:], aps["wv16"][:])
    wsb = {}
    for pr in range(NPAIRS):
        wsb[("q", pr)] = wqk8[:, pr, 0]
        wsb[("k", pr)] = wqk8[:, pr, 1]
        wsb[("v", pr)] = wv16[:, pr]
    wosb = wpool.tile([128, 2, 1024], F16, tag="w", name="wo")
    nc.scalar.dma_start(wosb[:], aps["wo"][:])

    # ---- x: per seq-half tiles, one contiguous DMA each ----
    xsb = {}
    xbf = {}
    for h in (1, 0):   # h=1 halves feed the first three groups
        xt = xpool.tile([128, MKD, 2, 1024], FP8, tag="x", name=f"x8_{h}")
        nc.sync.dma_start(xt[:], aps["x8"][h])
        xsb[h] = xt
        xt = xpool.tile([128, 8, 1024], BF16, tag="xb", name=f"xb{h}")
        nc.sync.dma_start(xt[:], aps["xbf"][h])
        xbf[h] = xt

    qts, kts = {}, {}
    vas = {}
    BIDX = {"q": 0, "k": 1, "v": 2}

    # persistent vts staging tiles (ones row written once, reused)
    vts_tiles = []
    for i in range(4):
        vt = vtpool.tile([65, 512], BF16, tag="vt", name=f"vts{i}")
        nc.vector.memset(vt[64:65, :], 1.0)
        vts_tiles.append(vt)
    _vts_ctr = [0]

    for pr in range(NPAIRS):
        qt = qkpool.tile([128, 2048], F16, tag="qk", name=f"qT{pr}")
        kt = qkpool.tile([128, 2048], F16, tag="qk", name=f"kT{pr}")
        qts[pr], kts[pr] = qt, kt
        for h01 in range(2):
            va = vapool.tile([128, PC, 2, 80], FP8, tag="va",
                             name=f"va{pr}{h01}")
            vas[(pr, h01)] = va

    def proj_group(t, pr, h, ps_pool, pst_pool, seq=False):
        """One projection group: matmul of type t, pair pr, seq half h
        (pq columns [1024h, 1024h+1024)) into two [128,512] PSUM tiles
        (or one at a time when seq=True, for a 1-buffer ring); evacuate
        on DVE with the bias fold."""
        w = wsb[(t, pr)]

        def mm(pp, d):
            if t in ("q", "k"):
                for c in range(MKD):
                    nc.tensor.matmul(
                        pp[:],
                        w[:, c, :, :],
                        xsb[h][:, c, :, 512 * d:512 * (d + 1)],
                        start=(c == 0), stop=(c == MKD - 1),
                        perf_mode=DR,
                    )
            else:
                for mk in range(8):
                    nc.tensor.matmul(
                        pp[:],
                        w[:, mk, :],
                        xbf[h][:, mk, 512 * d:512 * (d + 1)],
                        start=(mk == 0), stop=(mk == 7),
                    )

        bias = bcol[:, BIDX[t] + 3 * pr:BIDX[t] + 3 * pr + 1]
        if t in ("q", "k"):
            dest = (qts if t == "q" else kts)[pr]
            for d in ((1, 0) if h == 1 else (0, 1)):
                pp = ps_pool.tile([128, 512], F32, tag="aux",
                                  name=f"prj_{t}{pr}{h}{d}")
                mm(pp, d)
                nc.vector.tensor_scalar_add(
                    dest[:, 1024 * h + 512 * d:1024 * h + 512 * (d + 1)],
                    pp[:], bias,
                )
        else:
            # v: per d-tile, evacuate both heads to vts staging, then
            # PE-transpose to [128 pk, 65] and cast into the fp8 va planes
            units = []
            for d in range(2):
                j4 = 2 * h + d
                pp = ps_pool.tile([128, 512], F32, tag="aux",
                                  name=f"prj_v{pr}{h}{d}")
                mm(pp, d)
                for h01 in range(2):
                    vt = vts_tiles[_vts_ctr[0] % 4]
                    _vts_ctr[0] += 1
                    nc.vector.tensor_scalar_add(
                        vt[0:64, :],
                        pp[64 * h01:64 * (h01 + 1), :],
                        bcol[64 * h01:64 * (h01 + 1),
                             BIDX[t] + 3 * pr:BIDX[t] + 3 * pr + 1],
                    )
                    units.append((vt, h01, j4))
            for vt, h01, j4 in units:
                pst = pst_pool.tile([128, 4, 66], BF16, tag="aux",
                                   name=f"pst{pr}{h01}{j4}")
                for c4 in range(4):
                    nc.tensor.transpose(
                        pst[:, c4, 0:65],
                        vt[:, 128 * c4:128 * (c4 + 1)],
                        eyebf[0:65, 0:65],
                    )
                vhi = vas[(pr, h01)][:, 4 * j4:4 * j4 + 4, 0, 0:65]
                nc.vector.tensor_copy(vhi, pst[:, :, 0:65])
                nc.vector.tensor_sub(
                    vas[(pr, h01)][:, 4 * j4:4 * j4 + 4, 1, 0:65],
                    pst[:, :, 0:65], vhi,
                )

    def attn_pair(pr, j, sps_pool, zps_pool, prev_finish=None):
        """Attention for head-pair pr, pq tile j: bf16 row-packed scores
        with PE-additive triangular mask, one exp per pk chunk (both
        heads), fp8 hi/lo-residual DoubleRow z accumulation, then
        broadcast-normalize into the fp16 zp tile (plane pr).

        The previous unit's z-drain + normalize (prev_finish) is emitted
        after this unit's first PRE score chunks so the in-order PE
        stream never stalls on the normalize chain; this unit's own
        drain is returned as a closure."""
        qt, kt = qts[pr], kts[pr]
        nchunk = PC - 4 * j
        PRE = min(2, nchunk)
        DW = min(4, nchunk)
        zpss = []
        descs = []
        state = {"zn": 0}

        def emit_z(zi):
            ex, i_, wp = descs[zi]
            for h01 in range(2):
                nc.tensor.matmul(
                    zpss[h01][:, 0:wp],
                    vas[(pr, h01)][:, i_, :, 0:65],
                    ex[:, 512 * h01:512 * h01 + wp]
                    .unsqueeze(1).broadcast_to((128, 2, wp)),
                    start=(zi == 0), stop=(zi == nchunk - 1),
                    perf_mode=DR,
                )
            state["zn"] = zi + 1

        def finish():
            for zi in range(state["zn"], nchunk):
                emit_z(zi)
            zpj = zp_tiles[j]
            for h01 in range(2):
                if j == PT - 1:
                    # fully-masked query row P-1: denom 0 -> 1
                    nc.vector.memset(zpss[h01][64:65, 511:512], 1.0)
                dsb = rcpool.tile([1, 512], F32, tag="dn")
                nc.vector.tensor_copy(dsb[:], zpss[h01][64:65, :])
                rcp = rcpool.tile([1, 512], F32, tag="rc")
                nc.vector.reciprocal_approx_fast(rcp[:], dsb[:])
                bc = bcpool.tile([64, 512], F32, tag="bc")
                nc.gpsimd.partition_broadcast(bc[:], rcp[:])
                nc.vector.tensor_mul(
                    zpj[64 * h01:64 * (h01 + 1), pr, :],
                    zpss[h01][0:64, :], bc[:],
                )

        for idx in range(nchunk):
            if idx == PRE:
                if prev_finish is not None:
                    prev_finish()
                for h01 in range(2):
                    zpss.append(zps_pool.tile(
                        [65, 512], F32, tag="zps",
                        name=f"zps{pr}{j}{h01}"))
            i_ = PC - 1 - idx              # descending pk chunks
            tt = i_ - 4 * j
            wp = min(512, 128 * (tt + 1))
            ex = expool.tile([128, 1024], FP8, tag="ex")
            sps = sps_pool.tile([128, 1024], F32, tag="sc")
            for h01 in range(2):
                rows = slice(64 * h01, 64 * (h01 + 1))
                nc.tensor.matmul(
                    sps[:, 512 * h01:512 * h01 + wp],
                    kt[rows, 128 * i_:128 * (i_ + 1)],
                    qt[rows, 512 * j:512 * j + wp],
                    start=True, stop=(tt >= 4),
                )
            if tt < 4:
                nc.tensor.matmul(
                    sps[:].rearrange(
                        "p (two f) -> p two f",
                        two=2)[:, :, 128 * tt:128 * tt + 128],
                    eyebf[:],
                    maskneg[:, 0:128]
                    .unsqueeze(1).broadcast_to((128, 2, 128)),
                    start=False, stop=True,
                    skip_group_check=True,
                )
            nc.scalar.activation(
                ex[:].rearrange(
                    "p (two f) -> p two f", two=2)[:, :, 0:wp],
                sps[:].rearrange(
                    "p (two f) -> p two f", two=2)[:, :, 0:wp],
                EXP, scale=0.125,
            )
            descs.append((ex, i_, wp))
            zi = idx - DW
            if zi >= 0 and idx >= PRE:
                emit_z(zi)
        return finish

    def outproj(j, ps_pool):
        """Output projection for pq tile j: fp8 DR over both pairs."""
        zpj = zp_tiles[j]
        for c4 in range(4):
            ck = 4 * j + c4
            osb = opool.tile([128, 1024], F16, tag="osb")
            pps = [ps_pool.tile([128, 512], F32, tag="aux",
                               name=f"op{ck}{mt}") for mt in range(2)]
            for pr in range(2):
                for mt in range(2):
                    nc.tensor.matmul(
                        pps[mt][:],
                        zpj[:, pr, 128 * c4:128 * (c4 + 1)],
                        wosb[:, pr, 512 * mt:512 * (mt + 1)],
                        start=(pr == 0), stop=(pr == 1),
                    )
            for mt in range(2):
                nc.vector.tensor_copy(
                    osb[:, 512 * mt:512 * (mt + 1)], pps[mt][:])
            nc.sync.dma_start(outp[128 * ck:128 * (ck + 1), :], osb[:])

    zp_tiles = {}
    for j in range(PT):
        zp_tiles[j] = zppool.tile([128, 2, 512], F16, tag="zp",
                                  name=f"zp{j}")

    with tc.tile_pool(name="ps_m", bufs=2, space="PSUM") as ps_m, \
         tc.tile_pool(name="ps_zps", bufs=2, space="PSUM") as ps_zps, \
         tc.tile_pool(name="ps_aux", bufs=2, space="PSUM") as ps_aux:
        # h=1 halves first; j=3 units carry no inline z, so scores can
        # start before the v projection lands. Each unit's z-drain +
        # normalize (+ output projection) rides inside the next unit.
        def opfin(f, jj):
            def g():
                f()
                outproj(jj, ps_aux)
            return g

        # warm the PE p-state while the x DMA lands: dummy matmuls on
        # memset tiles (no DMA dependency)
        wz = consts.tile([128, 128], BF16, tag="warm", name="warmw")
        nc.vector.memset(wz[:], 0.0)
        wx = consts.tile([128, 512], BF16, tag="warm2", name="warmx")
        nc.vector.memset(wx[:], 0.0)
        for wi in range(20):
            wp_ = ps_m.tile([128, 512], F32, tag="sc", name=f"warm{wi}")
            nc.tensor.matmul(wp_[:], wz[:], wx[:], start=True, stop=True)
        proj_group("q", 0, 1, ps_aux, ps_aux)
        proj_group("k", 0, 1, ps_aux, ps_aux)
        fin = attn_pair(0, 3, ps_m, ps_zps)
        proj_group("v", 0, 1, ps_aux, ps_aux)
        fin = attn_pair(0, 2, ps_m, ps_zps, fin)
        proj_group("q", 0, 0, ps_aux, ps_aux)
        proj_group("k", 0, 0, ps_aux, ps_aux)
        proj_group("v", 0, 0, ps_aux, ps_aux)
        fin = attn_pair(0, 1, ps_m, ps_zps, fin)
        proj_group("q", 1, 1, ps_aux, ps_aux)
        proj_group("k", 1, 1, ps_aux, ps_aux)
        fin = attn_pair(0, 0, ps_m, ps_zps, fin)
        proj_group("v", 1, 1, ps_aux, ps_aux)
        fin = attn_pair(1, 3, ps_m, ps_zps, fin)
        proj_group("q", 1, 0, ps_aux, ps_aux)
        proj_group("k", 1, 0, ps_aux, ps_aux)
        fin = attn_pair(1, 2, ps_m, ps_zps, opfin(fin, 3))
        proj_group("v", 1, 0, ps_aux, ps_aux)
        fin = attn_pair(1, 1, ps_m, ps_zps, opfin(fin, 2))
        fin = attn_pair(1, 0, ps_m, ps_zps, opfin(fin, 1))
        fin()
        outproj(0, ps_aux)


def _build():
    if "v2" in _BUILT:
        return _BUILT["v2"]
    from contextlib import ExitStack

    nc = bacc.Bacc("TRN2", target_bir_lowering=False, debug=False)
    aps = {
        "x8": nc.dram_tensor("x8", [2, 128, MKD, 2, 1024], FP8,
                             kind="ExternalInput").ap(),
        "xbf": nc.dram_tensor("xbf", [2, 128, 8, 1024], BF16,
                              kind="ExternalInput").ap(),
        "wqk8": nc.dram_tensor("wqk8", [128, 2, 2, MKD, 2, 128], FP8,
                               kind="ExternalInput").ap(),
        "wv16": nc.dram_tensor("wv16", [128, 2, 8, 128], BF16,
                               kind="ExternalInput").ap(),
        "wo": nc.dram_tensor("wo", [128, 2, 1024], F16,
                             kind="ExternalInput").ap(),
        "bcol": nc.dram_tensor("bcol", [128, 6], F32,
                               kind="ExternalInput").ap(),
        "eyemask": nc.dram_tensor("eyemask", [128, 384], BF16,
                                  kind="ExternalInput").ap(),
        "outp": nc.dram_tensor("outp", [P, M], F16,
                               kind="ExternalOutput").ap(),
    }
    with tile.TileContext(nc) as tc:
        with ExitStack() as ctx, nc.allow_low_precision(
            reason="fp8 softmax kernel; verified numerically vs reference"
        ):
            _emit(nc, tc, aps, ctx)
    nc.compile()
    _BUILT["v2"] = nc
    return nc


def _host_inputs(x, kq, kk, kv, ko, bq, bk, bv):
    r = np.arange(128)
    m1 = np.where(r[None, :] >= r[:, None], MASKC, 0.0)  # block: mask c >= r
    eyemask = np.concatenate(
        [np.eye(128, dtype=np.float32), m1, np.full((128, 128), MASKC)],
        axis=1,
    ).astype(NP_BF16)  # [128, 384]

    in_maps = []
    for c in range(NCORES):
        b, k4 = divmod(c, 4)
        heads = [4 * k4 + i for i in range(HPC)]
        xT = np.ascontiguousarray(x[b].T)  # [1024, 2048]
        # x8[h][p, c, i, t'] = xT[256c + 128i + p, 1024h + t']
        x4 = xT.reshape(MKD, 2, 128, 2048).transpose(2, 0, 1, 3)
        x8 = np.ascontiguousarray(
            x4.reshape(128, MKD, 2, 2, 1024).transpose(3, 0, 1, 2, 4)
        ).astype(NP_FP8)  # [2, 128, MKD, 2, 1024]
        # xbf[h][p, mk, t'] = xT[128mk + p, 1024h + t']
        xb = xT.reshape(8, 128, 2, 1024).transpose(2, 1, 0, 3)
        xbf = np.ascontiguousarray(xb).astype(NP_BF16)  # [2, 128, 8, 1024]

        def pairm(kern, pr):
            return np.concatenate(
                [kern[heads[2 * pr]], kern[heads[2 * pr + 1]]], axis=1
            )  # [1024, 128]

        wqk8 = np.empty((128, 2, 2, MKD, 2, 128), NP_FP8)
        wv16 = np.empty((128, 2, 8, 128), NP_BF16)
        for pr in range(NPAIRS):
            for ti, kern in ((0, kq), (1, kk)):
                pm = pairm(kern, pr)  # [1024, 128]
                wqk8[:, pr, ti] = pm.reshape(MKD, 2, 128, 128).transpose(
                    2, 0, 1, 3).astype(NP_FP8)
            pmv = pairm(kv, pr)
            wv16[:, pr] = pmv.reshape(8, 128, 128).transpose(
                1, 0, 2).astype(NP_BF16)

        wo = np.stack(
            [np.concatenate([ko[heads[0]], ko[heads[1]]], axis=0),
             np.concatenate([ko[heads[2]], ko[heads[3]]], axis=0)], axis=1
        ).astype(np.float16)  # [128, 2, 1024]

        bcol = np.zeros((128, 6), np.float32)
        for pr in range(NPAIRS):
            for idx, bias in ((0, bq), (1, bk), (2, bv)):
                bcol[:, idx + 3 * pr] = np.concatenate(
                    [bias[heads[2 * pr]], bias[heads[2 * pr + 1]]]
                )

        in_maps.append({
            "x8": x8, "xbf": xbf,
            "wqk8": wqk8, "wv16": wv16,
            "wo": wo, "bcol": bcol,
            "eyemask": eyemask,
        })
    return in_maps


def kernel(x, kernel_query, kernel_key, kernel_value, kernel_out,
           bias_query, bias_key, bias_value, bias_out, _trace=False):
    x = np.asarray(x, np.float32)
    kq = np.asarray(kernel_query, np.float32)
    kk = np.asarray(kernel_key, np.float32)
    kv = np.asarray(kernel_value, np.float32)
    ko = np.asarray(kernel_out, np.float32)
    bq = np.asarray(bias_query, np.float32)
    bk = np.asarray(bias_key, np.float32)
    bv = np.asarray(bias_value, np.float32)
    bo = np.asarray(bias_out, np.float32)

    nc = _build()
    in_maps = _host_inputs(x, kq, kk, kv, ko, bq, bk, bv)
    res = bass_utils.run_bass_kernel_spmd(
        nc, in_maps, core_ids=list(range(NCORES)), trace=_trace
    )
    out = np.zeros((B, P, M), np.float32)
    for c in range(NCORES):
        out[c // 4] += res.results[c]["outp"].astype(np.float32)
    out += bo[None, None, :]

    # patch fully-masked query row P-1: uniform attention = mean_k v
    for b in range(B):
        xbar = x[b].mean(axis=0, dtype=np.float64)  # [M]
        row = np.zeros(M, np.float64)
        for n in range(N):
            zrow = xbar @ kv[n].astype(np.float64) + bv[n].astype(np.float64)
            row += zrow @ ko[n].astype(np.float64)
        out[b, P - 1, :] = (row + bo.astype(np.float64)).astype(np.float32)

    if _trace:
        kernel._last_result = res
    return out


# revision 46
# speedup vs baseline: 1.2013x; 1.0032x over previous
"""Trainium2 Bass kernel for multi-head attention (B=2, P=2048, M=1024, N=16, H=64).

out = softmax(mask(x@Wq @ (x@Wk)^T / sqrt(H))) @ (x@Wv) @ Wo + biases,
with the module's strictly-upper-triangular keep mask (row P-1 fully masked).

Sharding: 8 cores = 2 batches x 4 head-groups. Core c handles batch c//4,
heads [4*(c%4), 4*(c%4)+4); the host sums the 4 partial fp16 output
projections per batch and patches the fully-masked query row P-1
analytically. Measured: ~168 us HW exec (baseline 246 us), max rel err
~1.81e-2.

Design (per core; ACT-exp and PE co-paced under the core util throttle):
  - q/k projections: fp8e4 DoubleRow matmuls (two 128-row K-chunks per
    pass; DR contracts both planes at 1 col/cycle). The v projection runs
    in bf16: fp8 v errors hit concentrated-attention rows at full
    strength and blow the 2e-2 gate.
  - scores^T [pk, pq] in fp16, the two heads row-packed in disjoint
    K=64 PE row groups writing one [128,1024] PSUM tile.
  - The triangular mask is ADDITIVE on the PE: one eye@maskneg matmul
    per diagonal chunk covers both heads via a stride-0 broadcast AP;
    exp then yields exact fp8 zeros (no DVE masking).
  - One ACT exp per pk chunk covers both heads (strided [128,2,w] AP),
    writing fp8e4 ex directly (score range is ~e^{+-2.5}: safely inside
    fp8 with no max-subtraction).
  - z = v_aug^T @ ex^T accumulates via fp8 DoubleRow with v_aug stored
    as an fp8 hi+lo residual pair in the two DR planes (v = hi + lo to
    ~0.4%) and ex duplicated across planes by a stride-0 AP: 16-bit-
    grade v precision at fp8 cost. An appended ones column in v_aug
    yields the softmax denominators in PSUM row 64.
  - normalize: DVE copy of the denom row -> reciprocal_approx_fast ->
    gpsimd partition_broadcast -> one DVE multiply into the fp16 zp
    tile. No PE transposes anywhere in the z path.
  - output projection in fp16 (fp8 fails the error budget), both pairs
    accumulated per PSUM tile; osb evacuated on DVE, DMA'd as fp16.
  - Schedule: all input DMAs are consolidated (9 large contiguous
    transfers, h=1 seq-halves first); PE warmup matmuls hold the DVFS
    p-state through the DMA window; attention units are software-
    pipelined - each unit's z-drain + normalize (+ output projection
    for pair 1) is emitted inside the NEXT unit's score stream so the
    in-order engine queues never park behind the normalize chain;
    pair-1 projection groups are spread between attention units.
"""
import sys

import numpy as np

if "/opt/trn_rl_repo" not in sys.path:
    sys.path.insert(0, "/opt/trn_rl_repo")

import concourse.bacc as bacc
import concourse.tile as tile
from concourse import mybir
from concourse import bass_utils
import ml_dtypes

B, P, M, N, H = 2, 2048, 1024, 16, 64
NCORES = 8
HPC = 4          # heads per core
NPAIRS = 2       # head pairs per core
MKD = 4          # DoubleRow contraction chunks (256 each) for projections
PT = P // 512    # 4 pq tiles of 512
PC = P // 128    # 16 pk chunks of 128

F32 = mybir.dt.float32
F16 = mybir.dt.float16
BF16 = mybir.dt.bfloat16
FP8 = mybir.dt.float8e4
NP_FP8 = ml_dtypes.float8_e4m3
NP_BF16 = ml_dtypes.bfloat16
EXP = mybir.ActivationFunctionType.Exp
DR = mybir.MatmulPerfMode.DoubleRow
MASKC = -240.0   # exp(0.125 * -240) = e^-30 -> exact fp8 zero

_BUILT = {}


def _emit(nc, tc, aps, ctx):
    outp = aps["outp"]      # [2048, 1024] f16

    consts = ctx.enter_context(tc.tile_pool(name="consts", bufs=1))
    xpool = ctx.enter_context(tc.tile_pool(name="xpool", bufs=2))
    wpool = ctx.enter_context(tc.tile_pool(name="wpool", bufs=1))
    qkpool = ctx.enter_context(tc.tile_pool(name="qkpool", bufs=4))
    vtpool = ctx.enter_context(tc.tile_pool(name="vtpool", bufs=4))
    vapool = ctx.enter_context(tc.tile_pool(name="vapool", bufs=4))
    expool = ctx.enter_context(tc.tile_pool(name="expool", bufs=10))
    zppool = ctx.enter_context(tc.tile_pool(name="zppool", bufs=4))
    rcpool = ctx.enter_context(tc.tile_pool(name="rcpool", bufs=2))
    bcpool = ctx.enter_context(tc.tile_pool(name="bcpool", bufs=2))
    opool = ctx.enter_context(tc.tile_pool(name="opool", bufs=3))

    # ---- constants (one DMA: eye | maskneg) ----
    eyemask = consts.tile([128, 384], BF16)
    nc.scalar.dma_start(eyemask[:], aps["eyemask"][:])
    eyebf = eyemask[:, 0:128]
    maskneg = eyemask[:, 128:384]
    bcol = consts.tile([128, 6], F32)   # (q0,k0,v0,q1,k1,v1)
    nc.scalar.dma_start(bcol[:], aps["bcol"][:])

    # ---- weights: three DMAs ----
    wqk8 = wpool.tile([128, 2, 2, MKD, 2, 128], FP8, tag="wqk", name="wqk8")
    nc.scalar.dma_start(wqk8[:], aps["wqk8"][:])
    wv16 = wpool.tile([128, 2, 8, 128], BF16, tag="wv", name="wv16")
    nc.scalar.dma_start(wv16[:], aps["wv16"][:])
    wsb = {}
    for pr in range(NPAIRS):
        wsb[("q", pr)] = wqk8[:, pr, 0]
        wsb[("k", pr)] = wqk8[:, pr, 1]
        wsb[("v", pr)] = wv16[:, pr]
    wosb = wpool.tile([128, 2, 1024], F16, tag="w", name="wo")
    nc.scalar.dma_start(wosb[:], aps["wo"][:])

    # ---- x: per seq-half tiles, one contiguous DMA each ----
    xsb = {}
    xbf = {}
    for h in (1, 0):   # h=1 halves feed the first three groups
        xt = xpool.tile([128, MKD, 2, 1024], FP8, tag="x", name=f"x8_{h}")
        nc.sync.dma_start(xt[:], aps["x8"][h])
        xsb[h] = xt
        xt = xpool.tile([128, 8, 1024], BF16, tag="xb", name=f"xb{h}")
        nc.sync.dma_start(xt[:], aps["xbf"][h])
        xbf[h] = xt

    qts, kts = {}, {}
    vas = {}
    BIDX = {"q": 0, "k": 1, "v": 2}

    # persistent vts staging tiles (ones row written once, reused)
    vts_tiles = []
    for i in range(4):
        vt = vtpool.tile([65, 512], BF16, tag="vt", name=f"vts{i}")
        nc.vector.memset(vt[64:65, :], 1.0)
        vts_tiles.append(vt)
    _vts_ctr = [0]

    for pr in range(NPAIRS):
        qt = qkpool.tile([128, 2048], F16, tag="qk", name=f"qT{pr}")
        kt = qkpool.tile([128, 2048], F16, tag="qk", name=f"kT{pr}")
        qts[pr], kts[pr] = qt, kt
        for h01 in range(2):
            va = vapool.tile([128, PC, 2, 80], FP8, tag="va",
                             name=f"va{pr}{h01}")
            vas[(pr, h01)] = va

    def proj_group(t, pr, h, ps_pool, pst_pool, dsel=None):
        """One projection group: matmul of type t, pair pr, seq half h
        (pq columns [1024h, 1024h+1024)) into two [128,512] PSUM tiles
        (or one at a time when seq=True, for a 1-buffer ring); evacuate
        on DVE with the bias fold."""
        w = wsb[(t, pr)]

        def mm(pp, d):
            if t in ("q", "k"):
                for c in range(MKD):
                    nc.tensor.matmul(
                        pp[:],
                        w[:, c, :, :],
                        xsb[h][:, c, :, 512 * d:512 * (d + 1)],
                        start=(c == 0), stop=(c == MKD - 1),
                        perf_mode=DR,
                    )
            else:
                for mk in range(8):
                    nc.tensor.matmul(
                        pp[:],
                        w[:, mk, :],
                        xbf[h][:, mk, 512 * d:512 * (d + 1)],
                        start=(mk == 0), stop=(mk == 7),
                    )

        bias = bcol[:, BIDX[t] + 3 * pr:BIDX[t] + 3 * pr + 1]
        if t in ("q", "k"):
            dest = (qts if t == "q" else kts)[pr]
            for d in (dsel if dsel is not None
                      else ((1, 0) if h == 1 else (0, 1))):
                pp = ps_pool.tile([128, 512], F32, tag="aux",
                                  name=f"prj_{t}{pr}{h}{d}")
                mm(pp, d)
                nc.vector.tensor_scalar_add(
                    dest[:, 1024 * h + 512 * d:1024 * h + 512 * (d + 1)],
                    pp[:], bias,
                )
        else:
            # v: per d-tile, evacuate both heads to vts staging, then
            # PE-transpose to [128 pk, 65] and cast into the fp8 va planes
            units = []
            for d in range(2):
                j4 = 2 * h + d
                pp = ps_pool.tile([128, 512], F32, tag="aux",
                                  name=f"prj_v{pr}{h}{d}")
                mm(pp, d)
                for h01 in range(2):
                    vt = vts_tiles[_vts_ctr[0] % 4]
                    _vts_ctr[0] += 1
                    nc.vector.tensor_scalar_add(
                        vt[0:64, :],
                        pp[64 * h01:64 * (h01 + 1), :],
                        bcol[64 * h01:64 * (h01 + 1),
                             BIDX[t] + 3 * pr:BIDX[t] + 3 * pr + 1],
                    )
                    units.append((vt, h01, j4))
            for vt, h01, j4 in units:
                pst = pst_pool.tile([128, 4, 66], BF16, tag="aux",
                                   name=f"pst{pr}{h01}{j4}")
                for c4 in range(4):
                    nc.tensor.transpose(
                        pst[:, c4, 0:65],
                        vt[:, 128 * c4:128 * (c4 + 1)],
                        eyebf[0:65, 0:65],
                    )
                vhi = vas[(pr, h01)][:, 4 * j4:4 * j4 + 4, 0, 0:65]
                nc.vector.tensor_copy(vhi, pst[:, :, 0:65])
                nc.vector.tensor_sub(
                    vas[(pr, h01)][:, 4 * j4:4 * j4 + 4, 1, 0:65],
                    pst[:, :, 0:65], vhi,
                )

    def attn_pair(pr, j, sps_pool, zps_pool, prev_finish=None):
        """Attention for head-pair pr, pq tile j: bf16 row-packed scores
        with PE-additive triangular mask, one exp per pk chunk (both
        heads), fp8 hi/lo-residual DoubleRow z accumulation, then
        broadcast-normalize into the fp16 zp tile (plane pr).

        The previous unit's z-drain + normalize (prev_finish) is emitted
        after this unit's first PRE score chunks so the in-order PE
        stream never stalls on the normalize chain; this unit's own
        drain is returned as a closure."""
        qt, kt = qts[pr], kts[pr]
        nchunk = PC - 4 * j
        PRE = min(2, nchunk)
        DW = min(4, nchunk)
        zpss = []
        descs = []
        state = {"zn": 0}

        def emit_z(zi):
            ex, i_, wp = descs[zi]
            for h01 in range(2):
                nc.tensor.matmul(
                    zpss[h01][:, 0:wp],
                    vas[(pr, h01)][:, i_, :, 0:65],
                    ex[:, 512 * h01:512 * h01 + wp]
                    .unsqueeze(1).broadcast_to((128, 2, wp)),
                    start=(zi == 0), stop=(zi == nchunk - 1),
                    perf_mode=DR,
                )
            state["zn"] = zi + 1

        def finish():
            for zi in range(state["zn"], nchunk):
                emit_z(zi)
            zpj = zp_tiles[j]
            for h01 in range(2):
                if j == PT - 1:
                    # fully-masked query row P-1: denom 0 -> 1
                    nc.vector.memset(zpss[h01][64:65, 511:512], 1.0)
                dsb = rcpool.tile([1, 512], F32, tag="dn")
                nc.vector.tensor_copy(dsb[:], zpss[h01][64:65, :])
                rcp = rcpool.tile([1, 512], F32, tag="rc")
                nc.vector.reciprocal_approx_fast(rcp[:], dsb[:])
                bc = bcpool.tile([64, 512], F32, tag="bc")
                nc.gpsimd.partition_broadcast(bc[:], rcp[:])
                nc.vector.tensor_mul(
                    zpj[64 * h01:64 * (h01 + 1), pr, :],
                    zpss[h01][0:64, :], bc[:],
                )

        for idx in range(nchunk):
            if idx == PRE:
                if prev_finish is not None:
                    prev_finish()
                for h01 in range(2):
                    zpss.append(zps_pool.tile(
                        [65, 512], F32, tag="zps",
                        name=f"zps{pr}{j}{h01}"))
            i_ = PC - 1 - idx              # descending pk chunks
            tt = i_ - 4 * j
            wp = min(512, 128 * (tt + 1))
            ex = expool.tile([128, 1024], FP8, tag="ex")
            sps = sps_pool.tile([128, 1024], F32, tag="sc")
            for h01 in range(2):
                rows = slice(64 * h01, 64 * (h01 + 1))
                nc.tensor.matmul(
                    sps[:, 512 * h01:512 * h01 + wp],
                    kt[rows, 128 * i_:128 * (i_ + 1)],
                    qt[rows, 512 * j:512 * j + wp],
                    start=True, stop=(tt >= 4),
                )
            if tt < 4:
                nc.tensor.matmul(
                    sps[:].rearrange(
                        "p (two f) -> p two f",
                        two=2)[:, :, 128 * tt:128 * tt + 128],
                    eyebf[:],
                    maskneg[:, 0:128]
                    .unsqueeze(1).broadcast_to((128, 2, 128)),
                    start=False, stop=True,
                    skip_group_check=True,
                )
            nc.scalar.activation(
                ex[:].rearrange(
                    "p (two f) -> p two f", two=2)[:, :, 0:wp],
                sps[:].rearrange(
                    "p (two f) -> p two f", two=2)[:, :, 0:wp],
                EXP, scale=0.125,
            )
            descs.append((ex, i_, wp))
            zi = idx - DW
            if zi >= 0 and idx >= PRE:
                emit_z(zi)
        return finish

    def outproj(j, ps_pool):
        """Output projection for pq tile j: fp8 DR over both pairs."""
        zpj = zp_tiles[j]
        for c4 in range(4):
            ck = 4 * j + c4
            osb = opool.tile([128, 1024], F16, tag="osb")
            pps = [ps_pool.tile([128, 512], F32, tag="aux",
                               name=f"op{ck}{mt}") for mt in range(2)]
            for pr in range(2):
                for mt in range(2):
                    nc.tensor.matmul(
                        pps[mt][:],
                        zpj[:, pr, 128 * c4:128 * (c4 + 1)],
                        wosb[:, pr, 512 * mt:512 * (mt + 1)],
                        start=(pr == 0), stop=(pr == 1),
                    )
            for mt in range(2):
                nc.vector.tensor_copy(
                    osb[:, 512 * mt:512 * (mt + 1)], pps[mt][:])
            nc.sync.dma_start(outp[128 * ck:128 * (ck + 1), :], osb[:])

    zp_tiles = {}
    for j in range(PT):
        zp_tiles[j] = zppool.tile([128, 2, 512], F16, tag="zp",
                                  name=f"zp{j}")

    with tc.tile_pool(name="ps_m", bufs=2, space="PSUM") as ps_m, \
         tc.tile_pool(name="ps_zps", bufs=2, space="PSUM") as ps_zps, \
         tc.tile_pool(name="ps_aux", bufs=2, space="PSUM") as ps_aux:
        # h=1 halves first; j=3 units carry no inline z, so scores can
        # start before the v projection lands. Each unit's z-drain +
        # normalize (+ output projection) rides inside the next unit.
        def opfin(f, jj):
            def g():
                f()
                outproj(jj, ps_aux)
            return g

        # warm the PE p-state while the x DMA lands: dummy matmuls on
        # memset tiles (no DMA dependency)
        wz = consts.tile([128, 128], BF16, tag="warm", name="warmw")
        nc.vector.memset(wz[:], 0.0)
        wx = consts.tile([128, 512], BF16, tag="warm2", name="warmx")
        nc.vector.memset(wx[:], 0.0)
        for wi in range(20):
            wp_ = ps_m.tile([128, 512], F32, tag="sc", name=f"warm{wi}")
            nc.tensor.matmul(wp_[:], wz[:], wx[:], start=True, stop=True)
        proj_group("q", 0, 1, ps_aux, ps_aux, dsel=(1,))
        proj_group("k", 0, 1, ps_aux, ps_aux, dsel=(1,))
        fin = attn_pair(0, 3, ps_m, ps_zps)
        proj_group("q", 0, 1, ps_aux, ps_aux, dsel=(0,))
        proj_group("k", 0, 1, ps_aux, ps_aux, dsel=(0,))
        proj_group("v", 0, 1, ps_aux, ps_aux)
        fin = attn_pair(0, 2, ps_m, ps_zps, fin)
        proj_group("q", 0, 0, ps_aux, ps_aux)
        proj_group("k", 0, 0, ps_aux, ps_aux)
        proj_group("v", 0, 0, ps_aux, ps_aux)
        fin = attn_pair(0, 1, ps_m, ps_zps, fin)
        proj_group("q", 1, 1, ps_aux, ps_aux)
        proj_group("k", 1, 1, ps_aux, ps_aux)
        fin = attn_pair(0, 0, ps_m, ps_zps, fin)
        proj_group("v", 1, 1, ps_aux, ps_aux)
        fin = attn_pair(1, 3, ps_m, ps_zps, fin)
        proj_group("q", 1, 0, ps_aux, ps_aux)
        proj_group("k", 1, 0, ps_aux, ps_aux)
        fin = attn_pair(1, 2, ps_m, ps_zps, opfin(fin, 3))
        proj_group("v", 1, 0, ps_aux, ps_aux)
        fin = attn_pair(1, 1, ps_m, ps_zps, opfin(fin, 2))
        fin = attn_pair(1, 0, ps_m, ps_zps, opfin(fin, 1))
        fin()
        outproj(0, ps_aux)


def _build():
    if "v2" in _BUILT:
        return _BUILT["v2"]
    from contextlib import ExitStack

    nc = bacc.Bacc("TRN2", target_bir_lowering=False, debug=False)
    aps = {
        "x8": nc.dram_tensor("x8", [2, 128, MKD, 2, 1024], FP8,
                             kind="ExternalInput").ap(),
        "xbf": nc.dram_tensor("xbf", [2, 128, 8, 1024], BF16,
                              kind="ExternalInput").ap(),
        "wqk8": nc.dram_tensor("wqk8", [128, 2, 2, MKD, 2, 128], FP8,
                               kind="ExternalInput").ap(),
        "wv16": nc.dram_tensor("wv16", [128, 2, 8, 128], BF16,
                               kind="ExternalInput").ap(),
        "wo": nc.dram_tensor("wo", [128, 2, 1024], F16,
                             kind="ExternalInput").ap(),
        "bcol": nc.dram_tensor("bcol", [128, 6], F32,
                               kind="ExternalInput").ap(),
        "eyemask": nc.dram_tensor("eyemask", [128, 384], BF16,
                                  kind="ExternalInput").ap(),
        "outp": nc.dram_tensor("outp", [P, M], F16,
                               kind="ExternalOutput").ap(),
    }
    with tile.TileContext(nc) as tc:
        with ExitStack() as ctx, nc.allow_low_precision(
            reason="fp8 softmax kernel; verified numerically vs reference"
        ):
            _emit(nc, tc, aps, ctx)
    nc.compile()
    _BUILT["v2"] = nc
    return nc


def _host_inputs(x, kq, kk, kv, ko, bq, bk, bv):
    r = np.arange(128)
    m1 = np.where(r[None, :] >= r[:, None], MASKC, 0.0)  # block: mask c >= r
    eyemask = np.concatenate(
        [np.eye(128, dtype=np.float32), m1, np.full((128, 128), MASKC)],
        axis=1,
    ).astype(NP_BF16)  # [128, 384]

    in_maps = []
    for c in range(NCORES):
        b, k4 = divmod(c, 4)
        heads = [4 * k4 + i for i in range(HPC)]
        xT = np.ascontiguousarray(x[b].T)  # [1024, 2048]
        # x8[h][p, c, i, t'] = xT[256c + 128i + p, 1024h + t']
        x4 = xT.reshape(MKD, 2, 128, 2048).transpose(2, 0, 1, 3)
        x8 = np.ascontiguousarray(
            x4.reshape(128, MKD, 2, 2, 1024).transpose(3, 0, 1, 2, 4)
        ).astype(NP_FP8)  # [2, 128, MKD, 2, 1024]
        # xbf[h][p, mk, t'] = xT[128mk + p, 1024h + t']
        xb = xT.reshape(8, 128, 2, 1024).transpose(2, 1, 0, 3)
        xbf = np.ascontiguousarray(xb).astype(NP_BF16)  # [2, 128, 8, 1024]

        def pairm(kern, pr):
            return np.concatenate(
                [kern[heads[2 * pr]], kern[heads[2 * pr + 1]]], axis=1
            )  # [1024, 128]

        wqk8 = np.empty((128, 2, 2, MKD, 2, 128), NP_FP8)
        wv16 = np.empty((128, 2, 8, 128), NP_BF16)
        for pr in range(NPAIRS):
            for ti, kern in ((0, kq), (1, kk)):
                pm = pairm(kern, pr)  # [1024, 128]
                wqk8[:, pr, ti] = pm.reshape(MKD, 2, 128, 128).transpose(
                    2, 0, 1, 3).astype(NP_FP8)
            pmv = pairm(kv, pr)
            wv16[:, pr] = pmv.reshape(8, 128, 128).transpose(
                1, 0, 2).astype(NP_BF16)

        wo = np.stack(
            [np.concatenate([ko[heads[0]], ko[heads[1]]], axis=0),
             np.concatenate([ko[heads[2]], ko[heads[3]]], axis=0)], axis=1
        ).astype(np.float16)  # [128, 2, 1024]

        bcol = np.zeros((128, 6), np.float32)
        for pr in range(NPAIRS):
            for idx, bias in ((0, bq), (1, bk), (2, bv)):
                bcol[:, idx + 3 * pr] = np.concatenate(
                    [bias[heads[2 * pr]], bias[heads[2 * pr + 1]]]
                )

        in_maps.append({
            "x8": x8, "xbf": xbf,
            "wqk8": wqk8, "wv16": wv16,
            "wo": wo, "bcol": bcol,
            "eyemask": eyemask,
        })
    return in_maps


def kernel(x, kernel_query, kernel_key, kernel_value, kernel_out,
           bias_query, bias_key, bias_value, bias_out, _trace=False):
    x = np.asarray(x, np.float32)
    kq = np.asarray(kernel_query, np.float32)
    kk = np.asarray(kernel_key, np.float32)
    kv = np.asarray(kernel_value, np.float32)
    ko = np.asarray(kernel_out, np.float32)
    bq = np.asarray(bias_query, np.float32)
    bk = np.asarray(bias_key, np.float32)
    bv = np.asarray(bias_value, np.float32)
    bo = np.asarray(bias_out, np.float32)

    nc = _build()
    in_maps = _host_inputs(x, kq, kk, kv, ko, bq, bk, bv)
    res = bass_utils.run_bass_kernel_spmd(
        nc, in_maps, core_ids=list(range(NCORES)), trace=_trace
    )
    out = np.zeros((B, P, M), np.float32)
    for c in range(NCORES):
        out[c // 4] += res.results[c]["outp"].astype(np.float32)
    out += bo[None, None, :]

    # patch fully-masked query row P-1: uniform attention = mean_k v
    for b in range(B):
        xbar = x[b].mean(axis=0, dtype=np.float64)  # [M]
        row = np.zeros(M, np.float64)
        for n in range(N):
            zrow = xbar @ kv[n].astype(np.float64) + bv[n].astype(np.float64)
            row += zrow @ ko[n].astype(np.float64)
        out[b, P - 1, :] = (row + bo.astype(np.float64)).astype(np.float32)

    if _trace:
        kernel._last_result = res
    return out
